# revision 1
# baseline (speedup 1.0000x reference)
"""Gemma3 sliding-window attention kernel for 8 Trainium2 NeuronCores.

Sharding: core c handles batch b = c//4, query-row chunk j = c%4 (512 rows).
The reference keeps only the LAST 512 key columns for every query row, so
each core computes k/v projections for rows 1536:2048 of its batch — all 4
kv heads locally (no collectives; the duplicated kv compute is cheaper than
the AllGather latency on hardware).

All matmul operands stream from HBM in bf16 (cast host-side); PSUM
accumulation is fp32 and softmax math stays fp32.

The attention works in HEAD PAIRS (each q-head pair shares one kv head, so
scores/softmax operands and the tanh scale coincide): DVE/ACT chain ops run
on [128, 2*512] pair tiles, halving the per-op overhead that dominates on
hardware. RMS normalization is deferred off the critical path: khat/qhat
hold rope((1+w)*raw); rs_k folds into the softcap tanh's per-partition
scale AP; rs_q is applied to qhat one pipeline step later from a
PE-broadcast row of sums of squares, with rsqrt done as a cubic seed + two
Newton steps (bf16 then fp32). Pipeline: step s issues qproj(pair s),
scores(s-2), attn_out(s-3).
"""

import numpy as np
import ml_dtypes

import concourse.bacc as bacc
import concourse.tile as tile
from concourse import mybir
from concourse.bass_utils import run_bass_kernel_spmd

F32 = mybir.dt.float32
F32R = mybir.dt.float32r
BF16 = mybir.dt.bfloat16
AF = mybir.ActivationFunctionType
OP = mybir.AluOpType

B, L, HID = 2, 2048, 2560
NH, NKV, D = 8, 4, 256
NP = NH // 2       # head pairs; pair p = heads (2p, 2p+1), kv head p
W = 512            # effective kv window (last W positions of the sequence)
CH = 512           # query rows per core
NCORES = 8
KT = HID // 128    # 20 contraction tiles for the projections
EPS = 1e-6
SOFTCAP = 50.0
SCALE = D ** -0.5
ROPE_BASE = 10000.0
NPBF16 = ml_dtypes.bfloat16
# tanh input scale c folded into the rs_k rsqrt: tanh((c*rs_k) * (rs_q*s))
C0 = SCALE / SOFTCAP
# cubic minimax seed for rsqrt on t in [0.3, 3.2] (rel err 4.9%), then two
# Newton steps (first bf16, second fp32) -> ~1e-4
RSQ_P3, RSQ_P2, RSQ_P1, RSQ_P0 = (-0.11751866, 0.81282722,
                                  -1.93345784, 2.24612936)


def _build(loop_n=None):
    nc = bacc.Bacc("TRN2", target_bir_lowering=False, debug=False,
                   num_devices=NCORES)
    xq_d = nc.dram_tensor("xq", [128, KT, CH], BF16, kind="ExternalInput").ap()
    xkv_d = nc.dram_tensor("xkv", [128, KT, W], BF16, kind="ExternalInput").ap()
    qw_d = nc.dram_tensor("qw", [NP, 128, KT, 2 * D], BF16,
                          kind="ExternalInput").ap()
    kwa_d = nc.dram_tensor("kwa", [128, KT, 512], BF16, kind="ExternalInput").ap()
    kwb_d = nc.dram_tensor("kwb", [128, KT, 512], BF16, kind="ExternalInput").ap()
    vwa_d = nc.dram_tensor("vwa", [128, KT, 512], BF16, kind="ExternalInput").ap()
    vwb_d = nc.dram_tensor("vwb", [128, KT, 512], BF16, kind="ExternalInput").ap()
    ow_d = nc.dram_tensor("ow", [128, HID // 128, 16, 128], BF16,
                          kind="ExternalInput").ap()
    # rope tables duplicated over the pair dim for [128, 2, *] chain ops
    cq = nc.dram_tensor("cq", [128, 2, CH], BF16, kind="ExternalInput").ap()
    sq = nc.dram_tensor("sq", [128, 2, CH], BF16, kind="ExternalInput").ap()
    sqn = nc.dram_tensor("sqn", [128, 2, CH], BF16, kind="ExternalInput").ap()
    ck = nc.dram_tensor("ck", [128, 2, W], BF16, kind="ExternalInput").ap()
    sk = nc.dram_tensor("sk", [128, 2, W], BF16, kind="ExternalInput").ap()
    skn = nc.dram_tensor("skn", [128, 2, W], BF16, kind="ExternalInput").ap()
    # columns: 1+qnw[:128], 1+qnw[128:], 1+knw[:128], 1+knw[128:]
    w1p = nc.dram_tensor("w1p", [128, 4], F32, kind="ExternalInput").ap()
    onesc_d = nc.dram_tensor("onesc", [128, 1], BF16, kind="ExternalInput").ap()
    onesr_d = nc.dram_tensor("onesr", [1, 128], F32R, kind="ExternalInput").ap()
    yT = nc.dram_tensor("yT", [HID, CH], F32, kind="ExternalOutput").ap()

    NKC = 4
    CKT = KT // NKC

    with tile.TileContext(nc) as tc, \
            nc.allow_low_precision(reason='bf16 matmul operands'):
        with (
            tc.tile_pool(name="const", bufs=1) as pc,
            tc.tile_pool(name="px", bufs=2) as px,
            tc.tile_pool(name="pkw", bufs=2) as pkw,
            tc.tile_pool(name="pow", bufs=3) as pow_,
            tc.tile_pool(name="pkv", bufs=1) as pkv,
            tc.tile_pool(name="pq", bufs=1) as pq,
            tc.tile_pool(name="ptmp", bufs=2) as ptmp,
            tc.tile_pool(name="prow", bufs=1) as prow,
            tc.tile_pool(name="pexp", bufs=2) as pexp,
            tc.tile_pool(name="pout", bufs=2) as pout,
            tc.tile_pool(name="pp", bufs=3, space="PSUM") as pp,
        ):
            import contextlib
            loop_ctx = tc.For_i(0, loop_n, 1) if loop_n else contextlib.nullcontext()
            # constants
            ones_col = pc.tile([128, 1], BF16, tag="onesc")
            nc.scalar.dma_start(out=ones_col, in_=onesc_d)
            ones_row = pc.tile([1, 128], F32R, tag="onesr")
            nc.scalar.dma_start(out=ones_row, in_=onesr_d)
            ck_sb = pc.tile([128, 2, W], BF16, tag="c1")
            sk_sb = pc.tile([128, 2, W], BF16, tag="c2")
            skn_sb = pc.tile([128, 2, W], BF16, tag="c3")
            cq_sb = pc.tile([128, 2, CH], BF16, tag="c1")
            sq_sb = pc.tile([128, 2, CH], BF16, tag="c2")
            sqn_sb = pc.tile([128, 2, CH], BF16, tag="c3")
            w1p_sb = pc.tile([128, 4], F32, tag="w1p")
            nc.scalar.dma_start(out=w1p_sb, in_=w1p)
            rsk_sb = pc.tile([128, 4 * NKV], F32, tag="rsk")

            def rsqrt_sb(out_sb, t_sb, nfree, scale=1.0):
                """out = scale * t^-0.5 for SBUF f32 t (t in ~[0.3, 3.2]):
                cubic Horner seed + Newton iter in bf16, then one fp32
                Newton iter. y^2 goes through ACT Square."""
                z = ptmp.tile([128, nfree], BF16, tag="nwA", bufs=1)
                nc.vector.tensor_scalar(z, t_sb, RSQ_P3, RSQ_P2,
                                        op0=OP.mult, op1=OP.add)
                z2 = ptmp.tile([128, nfree], BF16, tag="nwB", bufs=1)
                nc.vector.scalar_tensor_tensor(z2, z, 0.0, t_sb,
                                               op0=OP.add, op1=OP.mult)
                z3 = ptmp.tile([128, nfree], BF16, tag="nwA", bufs=1)
                nc.vector.scalar_tensor_tensor(z3, z2, RSQ_P1, t_sb,
                                               op0=OP.add, op1=OP.mult)
                y = ptmp.tile([128, nfree], BF16, tag="nwB", bufs=1)
                nc.vector.tensor_scalar(y, z3, 1.0, RSQ_P0,
                                        op0=OP.mult, op1=OP.add)
                for it in range(2):
                    dt_ = BF16 if it == 0 else F32
                    sqy = ptmp.tile([128, nfree], dt_, tag="nwA", bufs=1,
                                    name=f"sqy{it}")
                    nc.scalar.square(sqy, y)
                    u = ptmp.tile([128, nfree], dt_, tag="nwC", bufs=1,
                                  name=f"nwu{it}")
                    nc.vector.tensor_mul(u, sqy, t_sb)
                    v = ptmp.tile([128, nfree], dt_, tag="nwA", bufs=1,
                                  name=f"nwv{it}")
                    nc.vector.tensor_scalar(v, u, -0.5, 1.5,
                                            op0=OP.mult, op1=OP.add)
                    if it == 0:
                        y1 = ptmp.tile([128, nfree], BF16, tag="nwD", bufs=1)
                        nc.vector.tensor_mul(y1, y, v)
                        y = y1
                    else:
                        nc.vector.scalar_tensor_tensor(out_sb, v, scale, y,
                                                       op0=OP.mult,
                                                       op1=OP.mult)

            def wrope_pair(ps0p, ps1p, hat, base, wcol0, wcol1, cos2, sin2,
                           nsin2, nfree):
                """rope((1+w) * raw) for a HEAD PAIR. ps0p/ps1p: PSUM
                [128, 2, nfree] (dim 1 = head in pair, halves split across
                the two tiles). Writes hat slots {base, base+2} (first
                halves) and {base+1, base+3} (second halves), bf16."""
                h0 = hat[:, base:base + 4:2, :]
                h1 = hat[:, base + 1:base + 4:2, :]
                # both ps0p reads first, then ps1p, so the PSUM banks free
                # as early as possible for the next consumer
                a = ptmp.tile([128, 2, nfree], BF16, tag="ra", bufs=1)
                b2 = ptmp.tile([128, 2, nfree], BF16, tag="rb", bufs=1)
                nc.vector.scalar_tensor_tensor(a, ps0p, wcol0, cos2,
                                               op0=OP.mult, op1=OP.mult)
                nc.vector.scalar_tensor_tensor(b2, ps0p, wcol0, sin2,
                                               op0=OP.mult, op1=OP.mult)
                bn = ptmp.tile([128, 2, nfree], BF16, tag="nwA", bufs=1)
                a2 = ptmp.tile([128, 2, nfree], BF16, tag="nwB", bufs=1)
                nc.vector.scalar_tensor_tensor(bn, ps1p, wcol1, nsin2,
                                               op0=OP.mult, op1=OP.mult)
                nc.vector.scalar_tensor_tensor(a2, ps1p, wcol1, cos2,
                                               op0=OP.mult, op1=OP.mult)
                nc.vector.tensor_add(h0, a, bn)
                nc.vector.tensor_add(h1, a2, b2)

            with loop_ctx:
                khat = pkv.tile([128, 2 * NKV, W], BF16, tag="khat")
                v_sb = pkv.tile([128, 4, NKV * D], BF16, tag="v")
                qhat = pq.tile([128, 2 * NH, CH], BF16, tag="qhat")

                # ---- Phase 1: local kv projection, all 4 heads ----
                xkv_sb = px.tile([128, KT, W], BF16, tag="x")
                kw_sb = [pkw.tile([128, KT, 512], BF16, tag="w",
                                  name=f"kw{wv}") for wv in range(2)]
                CHUNKS = [(0, 1), (1, 2), (2, 4), (4, 8), (8, 14), (14, 20)]
                for lo, hi in CHUNKS:
                    sl = slice(lo, hi)
                    nc.sync.dma_start(out=kw_sb[0][:, sl, :], in_=kwa_d[:, sl, :])
                    nc.sync.dma_start(out=xkv_sb[:, sl, :], in_=xkv_d[:, sl, :])
                for c in range(NKC):
                    sl = slice(c * CKT, (c + 1) * CKT)
                    nc.sync.dma_start(out=kw_sb[1][:, sl, :], in_=kwb_d[:, sl, :])
                nc.scalar.dma_start(out=ck_sb, in_=ck)
                nc.scalar.dma_start(out=sk_sb, in_=sk)
                nc.scalar.dma_start(out=skn_sb, in_=skn)

                # k projection: 2 waves = 2 head pairs.  kps[m][:, gl, :] is
                # head gl's half m.  ssT accumulates transposed sums of
                # squares so rs_k becomes a per-partition tanh scale.
                ssT = pp.tile([128, 16], F32, tag="b1", name="ssT", bufs=1)
                for wv in range(2):
                    kps = [pp.tile([128, 2, W], F32, tag="b2",
                                   name=f"kps{wv}{m}") for m in range(2)]
                    for kt in range(KT):
                        for gl in range(2):
                            for m in range(2):
                                nc.tensor.matmul(
                                    kps[m][:, gl, :],
                                    kw_sb[wv][:, kt,
                                              gl * 256 + m * 128:
                                              gl * 256 + (m + 1) * 128],
                                    xkv_sb[:, kt, :],
                                    start=(kt == 0), stop=(kt == KT - 1))
                    for gl in range(2):
                        sqk = [ptmp.tile([128, W], BF16, tag="tA",
                                         name=f"sqk{wv}{gl}{m}")
                               for m in range(2)]
                        for m in range(2):
                            nc.scalar.square(sqk[m], kps[m][:, gl, :])
                        for mlk in range(4):
                            idx = (2 * wv + gl) * 4 + mlk
                            for m in range(2):
                                nc.tensor.matmul(
                                    ssT[:, idx:idx + 1],
                                    sqk[m][:, mlk * 128:(mlk + 1) * 128],
                                    ones_col,
                                    start=(m == 0), stop=(m == 1))
                    wrope_pair(kps[0], kps[1], khat, 4 * wv,
                               w1p_sb[:, 2:3], w1p_sb[:, 3:4],
                               ck_sb, sk_sb, skn_sb, W)
                # rsk = c0 * (ssT/D + EPS)^-0.5 for all 4 kv heads at once
                tk = ptmp.tile([128, 16], F32, tag="tq", bufs=1, name="tk")
                nc.vector.tensor_scalar(tk, ssT, 1.0 / D, EPS,
                                        op0=OP.mult, op1=OP.add)
                rsqrt_sb(rsk_sb, tk, 16, scale=C0)

                # v projection: 2 waves x 2 heads
                vw_sb = [pkw.tile([128, KT, 512], BF16, tag="w",
                                  name=f"vw{wv}") for wv in range(2)]
                for wv, vd in ((0, vwa_d), (1, vwb_d)):
                    for c in range(NKC):
                        sl = slice(c * CKT, (c + 1) * CKT)
                        nc.sync.dma_start(out=vw_sb[wv][:, sl, :],
                                          in_=vd[:, sl, :])
                for wv in range(2):
                    vps = [pp.tile([128, 2, 512], F32, tag="b2",
                                   name=f"vps{wv}{mm2}") for mm2 in range(2)]
                    for kt in range(KT):
                        for m in range(4):
                            nc.tensor.matmul(
                                vps[m // 2][:, m % 2, :],
                                xkv_sb[:, kt, m * 128:(m + 1) * 128],
                                vw_sb[wv][:, kt, :],
                                start=(kt == 0), stop=(kt == KT - 1))
                    for mm2 in range(2):
                        # rows (2*mm2, 2*mm2+1) of the window chunk dim
                        nc.vector.tensor_copy(
                            v_sb[:, 2 * mm2:2 * mm2 + 2,
                                 wv * 512:(wv + 1) * 512],
                            vps[mm2])

                nc.sync.dma_start(out=cq_sb, in_=cq)
                nc.sync.dma_start(out=sq_sb, in_=sq)
                nc.sync.dma_start(out=sqn_sb, in_=sqn)

                # ---- Phase 2+3: pair pipeline  qproj(p) | scores(p-2) |
                #      attn_out(p-3) ----
                xq_sb = px.tile([128, KT, CH], BF16, tag="x")
                for c in range(NKC):
                    sl = slice(c * CKT, (c + 1) * CKT)
                    nc.sync.dma_start(out=xq_sb[:, sl, :], in_=xq_d[:, sl, :])
                aoT = px.tile([128, 2 * NH, CH], BF16, tag="x")

                qps_live = {}      # p -> [2 PSUM [128,2,CH] tiles (halves)]
                sqt_live = {}      # p -> [2 bf16 [128,2,CH] sq tiles]
                rbq_live = {}      # p -> rs_q broadcast [128,2,CH] f32
                exps_live = {}     # p -> exp tile [128, 4, 2, CH]
                dnrow_live = {}    # p -> [1, 2, CH] f32r
                qn_live = {}       # p -> normalized qhat [128, 4, CH] bf16
                qw_tiles = {}

                def qw_prefetch(p):
                    qw_t = pkw.tile([128, KT, 2 * D], BF16, tag="w",
                                    name=f"qwp{p}")
                    nc.sync.dma_start(out=qw_t, in_=qw_d[p])
                    qw_tiles[p] = qw_t

                def qproj_mms(p):
                    qw_t = qw_tiles.pop(p)
                    qps = [pp.tile([128, 2, CH], F32, tag="b2",
                                   name=f"qps{p}{m}") for m in range(2)]
                    for kt in range(KT):
                        for i in range(2):
                            for m in range(2):
                                nc.tensor.matmul(
                                    qps[m][:, i, :],
                                    qw_t[:, kt,
                                         i * 256 + m * 128:
                                         i * 256 + (m + 1) * 128],
                                    xq_sb[:, kt, :],
                                    start=(kt == 0), stop=(kt == KT - 1))
                    sqt = [ptmp.tile([128, 2, CH], BF16, tag="tA",
                                     name=f"sqt{p}{m}") for m in range(2)]
                    for m in range(2):
                        nc.scalar.square(sqt[m], qps[m])
                    qps_live[p] = qps
                    sqt_live[p] = sqt

                def rope_chain(p):
                    qps = qps_live.pop(p)
                    wrope_pair(qps[0], qps[1], qhat, 4 * p,
                               w1p_sb[:, 0:1], w1p_sb[:, 1:2],
                               cq_sb, sq_sb, sqn_sb, CH)

                def ss_mms(p):
                    """[1, 2, CH] rows of sums of squares for pair p."""
                    ss_ps = pp.tile([1, 2, CH], F32, tag="b2",
                                    name=f"ssq{p}")
                    sqt = sqt_live.pop(p)
                    for i in range(2):
                        for m in range(2):
                            nc.tensor.matmul(ss_ps[:, i, :], ones_col,
                                             sqt[m][:, i, :],
                                             start=(m == 0), stop=(m == 1))
                    ssrow = prow.tile([1, 2, CH], F32R, tag="row",
                                      name=f"ssrow{p}")
                    nc.scalar.copy(ssrow, ss_ps)
                    return ssrow

                def rbq_mms(p, ssrow):
                    rbq_ps = pp.tile([128, 2, CH], F32, tag="b2",
                                     name=f"rbq{p}")
                    for i in range(2):
                        nc.tensor.matmul(rbq_ps[:, i, :], ones_row,
                                         ssrow[:, i, :], start=True,
                                         stop=True)
                    return rbq_ps

                def newton_rbq(p, rbq_ps):
                    tq = ptmp.tile([128, 2, CH], F32, tag="tq", bufs=1,
                                   name=f"tq{p}")
                    nc.vector.tensor_scalar(tq, rbq_ps, 1.0 / D, EPS,
                                            op0=OP.mult, op1=OP.add)
                    rbq = ptmp.tile([128, 2, CH], F32, tag="rbB",
                                    name=f"rbqs{p}")
                    rsqrt_sb(rbq, tq, 2 * CH)
                    rbq_live[p] = rbq

                def qnorm_mul(p):
                    rbq = rbq_live.pop(p)
                    qn = pq.tile([128, 4, CH], BF16, tag="qhatn", bufs=2,
                                 name=f"qhatn{p}")
                    for i in range(2):
                        for dk in range(2):
                            nc.vector.tensor_mul(
                                qn[:, 2 * i + dk, :],
                                qhat[:, 4 * p + 2 * i + dk, :],
                                rbq[:, i, :])
                    qn_live[p] = qn

                def sps_softmax(p):
                    g = p
                    qn = qn_live.pop(p)
                    exps = pexp.tile([128, 4, 2, CH], BF16, tag="exps",
                                     name=f"exps{p}")
                    for mlk in range(4):
                        sps = pp.tile([128, 2, CH], F32, tag="b2",
                                      name=f"sps{p}{mlk}")
                        for i in range(2):
                            for dk in range(2):
                                nc.tensor.matmul(
                                    sps[:, i, :],
                                    khat[:, 2 * g + dk,
                                         mlk * 128:(mlk + 1) * 128],
                                    qn[:, 2 * i + dk, :],
                                    start=(dk == 0), stop=(dk == 1))
                        nc.scalar.activation(
                            sps, sps, AF.Tanh,
                            scale=rsk_sb[:, g * 4 + mlk:g * 4 + mlk + 1])
                        nc.scalar.activation(exps[:, mlk, :, :], sps, AF.Exp,
                                             scale=SOFTCAP)
                    exps_live[p] = exps

                def dn_part(p):
                    exps = exps_live[p]
                    dn_ps = pp.tile([1, 2, CH], F32, tag="b2",
                                    name=f"dn{p}")
                    for i in range(2):
                        for mlk in range(4):
                            nc.tensor.matmul(dn_ps[:, i, :], ones_col,
                                             exps[:, mlk, i, :],
                                             start=(mlk == 0),
                                             stop=(mlk == 3))
                    dnrow = prow.tile([1, 2, CH], F32R, tag="drow",
                                      name=f"dnrow{p}")
                    nc.scalar.copy(dnrow, dn_ps)
                    dnrow_live[p] = dnrow

                def ops_fin(p):
                    g = p
                    exps = exps_live.pop(p)
                    opst = []
                    for dh in range(2):
                        ops = pp.tile([128, 2, CH], F32, tag="b2",
                                      name=f"ops{p}{dh}")
                        for i in range(2):
                            for klk in range(4):
                                nc.tensor.matmul(
                                    ops[:, i, :],
                                    v_sb[:, klk,
                                         g * 256 + dh * 128:
                                         g * 256 + dh * 128 + 128],
                                    exps[:, klk, i, :],
                                    start=(klk == 0), stop=(klk == 3))
                        opst.append(ops)
                    rbat_ps = pp.tile([128, 2, CH], F32, tag="b2",
                                      name=f"rbat{p}")
                    dnrow = dnrow_live.pop(p)
                    for i in range(2):
                        nc.tensor.matmul(rbat_ps[:, i, :], ones_row,
                                         dnrow[:, i, :], start=True,
                                         stop=True)
                    rbat = ptmp.tile([128, 2, CH], F32, tag="rbB",
                                     name=f"rbat{p}")
                    nc.vector.reciprocal(rbat, rbat_ps)
                    for dh in range(2):
                        # aoT slots {4p+dh, 4p+2+dh}
                        nc.vector.tensor_mul(
                            aoT[:, 4 * p + dh:4 * p + dh + 3:2, :],
                            opst[dh], rbat)

                qw_prefetch(0)
                for s in range(NP + 3):
                    if 0 <= s - 3 < NP:
                        dn_part(s - 3)
                    if 0 <= s - 1 < NP:
                        rbq_ps = rbq_mms(s - 1, ss_mms(s - 1))
                    if 0 <= s - 3 < NP:
                        ops_fin(s - 3)
                    if 0 <= s - 2 < NP:
                        # consume rbq(s-2) before rbq(s-1) recycles its slot
                        qnorm_mul(s - 2)
                    if 0 <= s - 1 < NP:
                        newton_rbq(s - 1, rbq_ps)
                    if s + 1 < NP:
                        qw_prefetch(s + 1)
                    if s < NP:
                        qproj_mms(s)
                    if s < NP:
                        rope_chain(s)
                    if 0 <= s - 2 < NP:
                        sps_softmax(s - 2)

                # ---- Phase 4: o projection (outputs transposed: yT) ----
                for mp2 in range(HID // 256):
                    yps = pp.tile([128, 2, CH], F32, tag="b2",
                                  name=f"yps{mp2}")
                    for j in range(2):
                        mp = 2 * mp2 + j
                        owc = pow_.tile([128, 16, 128], BF16, tag="ow",
                                        name=f"ow{mp}")
                        nc.sync.dma_start(out=owc, in_=ow_d[:, mp, :, :])
                        for kk in range(16):
                            nc.tensor.matmul(yps[:, j, :], owc[:, kk, :],
                                             aoT[:, kk, :],
                                             start=(kk == 0), stop=(kk == 15))
                    yst = pout.tile([128, 2, CH], F32, tag="yst")
                    nc.scalar.copy(yst, yps)
                    for j in range(2):
                        nc.sync.dma_start(
                            out=yT[(2 * mp2 + j) * 128:
                                   (2 * mp2 + j + 1) * 128, :],
                            in_=yst[:, j, :])

    nc.compile()

    return nc


_NC_CACHE = {}


def _get_nc():
    if "nc" not in _NC_CACHE:
        _NC_CACHE["nc"] = _build()
    return _NC_CACHE["nc"]


def _rope_tables():
    inv_freq = 1.0 / (ROPE_BASE ** (np.arange(0, D, 2, dtype=np.float32) / D))
    t = np.arange(L, dtype=np.float32)
    freqs = np.outer(t, inv_freq)                     # (L, 128)
    return (np.ascontiguousarray(np.cos(freqs).T.astype(np.float32)),
            np.ascontiguousarray(np.sin(freqs).T.astype(np.float32)))


def _part_major(mat_t, free):
    """(HID_like, free) feature-major -> (128, KT_like, free) partition-major
    bf16 blocks: out[p, kt, f] = mat_t[kt*128 + p, f]."""
    r = mat_t.shape[0]
    return np.ascontiguousarray(
        mat_t.reshape(r // 128, 128, free).transpose(1, 0, 2).astype(NPBF16))


def _dup2(tab):
    """(128, N) f32 -> (128, 2, N) bf16 duplicated over dim 1."""
    return np.ascontiguousarray(
        np.repeat(tab[:, None, :], 2, axis=1).astype(NPBF16))


def _prep_in_maps(x, q_w, k_w, v_w, o_w, q_norm_w, k_norm_w):
    # q_w per pair: (128, KT, 512); feats = head_in_pair*256 + d
    qw_p = np.ascontiguousarray(
        q_w.reshape(NP, 2 * D, KT, 128).transpose(0, 3, 2, 1).astype(NPBF16))
    kwT = np.ascontiguousarray(k_w.T)                 # (HID, 1024)
    vwT = np.ascontiguousarray(v_w.T)
    kw_a = _part_major(kwT[:, :512], 512)
    kw_b = _part_major(kwT[:, 512:], 512)
    vw_a = _part_major(vwT[:, :512], 512)
    vw_b = _part_major(vwT[:, 512:], 512)
    # o_w: (128, 20, 16, 128); ow_p[p, mp, kk, f] = o_w[mp*128+f, kk*128+p]
    ow_p = np.ascontiguousarray(
        o_w.reshape(HID // 128, 128, 16, 128).transpose(3, 0, 2, 1)
        .astype(NPBF16))
    cosT, sinT = _rope_tables()                        # (128, L) each
    w1p = np.empty((128, 4), dtype=np.float32)
    w1p[:, 0] = 1.0 + q_norm_w[:128]
    w1p[:, 1] = 1.0 + q_norm_w[128:]
    w1p[:, 2] = 1.0 + k_norm_w[:128]
    w1p[:, 3] = 1.0 + k_norm_w[128:]

    kv_lo = L - W
    xkv_b = [_part_major(np.ascontiguousarray(x[b, kv_lo:, :].T), W)
             for b in range(B)]
    ck_t = _dup2(cosT[:, kv_lo:])
    sk_t = _dup2(sinT[:, kv_lo:])
    skn_t = _dup2(-sinT[:, kv_lo:])

    in_maps = []
    for c in range(NCORES):
        b, j = divmod(c, 4)
        rows = slice(j * CH, (j + 1) * CH)
        in_maps.append({
            "xq": _part_major(np.ascontiguousarray(x[b, rows, :].T), CH),
            "xkv": xkv_b[b],
            "qw": qw_p, "kwa": kw_a, "kwb": kw_b,
            "vwa": vw_a, "vwb": vw_b, "ow": ow_p,
            "cq": _dup2(cosT[:, rows]),
            "sq": _dup2(sinT[:, rows]),
            "sqn": _dup2(-sinT[:, rows]),
            "ck": ck_t, "sk": sk_t, "skn": skn_t,
            "w1p": w1p,
            "onesc": np.ones((128, 1), dtype=NPBF16),
            "onesr": np.ones((1, 128), dtype=np.float32),
        })
    return in_maps


def kernel(x, mask, q_w, k_w, v_w, o_w, q_norm_w, k_norm_w):
    x = np.asarray(x, dtype=np.float32)
    q_w = np.asarray(q_w, dtype=np.float32)
    k_w = np.asarray(k_w, dtype=np.float32)
    v_w = np.asarray(v_w, dtype=np.float32)
    o_w = np.asarray(o_w, dtype=np.float32)
    q_norm_w = np.asarray(q_norm_w, dtype=np.float32)
    k_norm_w = np.asarray(k_norm_w, dtype=np.float32)

    nc = _get_nc()
    in_maps = _prep_in_maps(x, q_w, k_w, v_w, o_w, q_norm_w, k_norm_w)

    res = run_bass_kernel_spmd(nc, in_maps, list(range(NCORES)))
    _NC_CACHE["last_res"] = res

    out = np.empty((B, L, HID), dtype=np.float32)
    for c in range(NCORES):
        b, j = divmod(c, 4)
        out[b, j * CH:(j + 1) * CH, :] = res.results[c]["yT"].T
    return out



# revision 13
# speedup vs baseline: 1.0340x; 1.0340x over previous
"""Gemma3 sliding-window attention kernel for 8 Trainium2 NeuronCores.

Sharding: core c handles batch b = c//4, query-row chunk j = c%4 (512 rows).
The reference keeps only the LAST 512 key columns for every query row, so
each core computes k/v projections for rows 1536:2048 of its batch — all 4
kv heads locally (no collectives; the duplicated kv compute is cheaper than
the AllGather latency on hardware).

All matmul operands stream from HBM in bf16 (cast host-side); PSUM
accumulation is fp32 and softmax math stays fp32.

The attention works in HEAD PAIRS (each q-head pair shares one kv head, so
scores/softmax operands and the tanh scale coincide): DVE/ACT chain ops run
on [128, 2*512] pair tiles, halving the per-op overhead that dominates on
hardware. RMS normalization is deferred off the critical path: khat/qhat
hold rope((1+w)*raw); rs_k folds into the softcap tanh's per-partition
scale AP; rs_q is applied to qhat one pipeline step later from a
PE-broadcast row of sums of squares, with rsqrt done as a cubic seed + two
Newton steps (bf16 then fp32). Pipeline: step s issues qproj(pair s),
scores(s-2), attn_out(s-3).
"""

import numpy as np
import ml_dtypes

import concourse.bacc as bacc
import concourse.tile as tile
from concourse import mybir
from concourse.bass_utils import run_bass_kernel_spmd

F32 = mybir.dt.float32
F32R = mybir.dt.float32r
BF16 = mybir.dt.bfloat16
AF = mybir.ActivationFunctionType
OP = mybir.AluOpType

B, L, HID = 2, 2048, 2560
NH, NKV, D = 8, 4, 256
NP = NH // 2       # head pairs; pair p = heads (2p, 2p+1), kv head p
W = 512            # effective kv window (last W positions of the sequence)
CH = 512           # query rows per core
NCORES = 8
KT = HID // 128    # 20 contraction tiles for the projections
EPS = 1e-6
SOFTCAP = 50.0
SCALE = D ** -0.5
ROPE_BASE = 10000.0
NPBF16 = ml_dtypes.bfloat16
# tanh input scale c folded into the rs_k rsqrt: tanh((c*rs_k) * (rs_q*s))
C0 = SCALE / SOFTCAP
# cubic minimax seed for rsqrt on t in [0.3, 3.2] (rel err 4.9%), then two
# Newton steps (first bf16, second fp32) -> ~1e-4
RSQ_P3, RSQ_P2, RSQ_P1, RSQ_P0 = (-0.11751866, 0.81282722,
                                  -1.93345784, 2.24612936)


def _build(loop_n=None):
    nc = bacc.Bacc("TRN2", target_bir_lowering=False, debug=False,
                   num_devices=NCORES)
    xq_d = nc.dram_tensor("xq", [128, KT, CH], BF16, kind="ExternalInput").ap()
    xkv_d = nc.dram_tensor("xkv", [128, KT, W], BF16, kind="ExternalInput").ap()
    qw_d = nc.dram_tensor("qw", [NP, 128, KT, 2 * D], BF16,
                          kind="ExternalInput").ap()
    kwa_d = nc.dram_tensor("kwa", [128, KT, 512], BF16, kind="ExternalInput").ap()
    kwb_d = nc.dram_tensor("kwb", [128, KT, 512], BF16, kind="ExternalInput").ap()
    vwa_d = nc.dram_tensor("vwa", [128, KT, 512], BF16, kind="ExternalInput").ap()
    vwb_d = nc.dram_tensor("vwb", [128, KT, 512], BF16, kind="ExternalInput").ap()
    ow_d = nc.dram_tensor("ow", [128, HID // 128, 16, 128], BF16,
                          kind="ExternalInput").ap()
    # rope tables duplicated over the pair dim for [128, 2, *] chain ops
    cq = nc.dram_tensor("cq", [128, 2, CH], BF16, kind="ExternalInput").ap()
    sq = nc.dram_tensor("sq", [128, 2, CH], BF16, kind="ExternalInput").ap()
    sqn = nc.dram_tensor("sqn", [128, 2, CH], BF16, kind="ExternalInput").ap()
    ck = nc.dram_tensor("ck", [128, 2, W], BF16, kind="ExternalInput").ap()
    sk = nc.dram_tensor("sk", [128, 2, W], BF16, kind="ExternalInput").ap()
    skn = nc.dram_tensor("skn", [128, 2, W], BF16, kind="ExternalInput").ap()
    # columns: 1+qnw[:128], 1+qnw[128:], 1+knw[:128], 1+knw[128:]
    w1p = nc.dram_tensor("w1p", [128, 4], F32, kind="ExternalInput").ap()
    onesc_d = nc.dram_tensor("onesc", [128, 1], BF16, kind="ExternalInput").ap()
    onesr_d = nc.dram_tensor("onesr", [1, 128], F32R, kind="ExternalInput").ap()
    ident_d = nc.dram_tensor("ident", [128, 128], F32R, kind="ExternalInput").ap()
    yT = nc.dram_tensor("yT", [HID, CH], F32, kind="ExternalOutput").ap()

    NKC = 4
    CKT = KT // NKC

    with tile.TileContext(nc) as tc, \
            nc.allow_low_precision(reason='bf16 matmul operands'):
        with (
            tc.tile_pool(name="const", bufs=1) as pc,
            tc.tile_pool(name="px", bufs=2) as px,
            tc.tile_pool(name="pkw", bufs=2) as pkw,
            tc.tile_pool(name="pow", bufs=3) as pow_,
            tc.tile_pool(name="pkv", bufs=1) as pkv,
            tc.tile_pool(name="pq", bufs=1) as pq,
            tc.tile_pool(name="ptmp", bufs=2) as ptmp,
            tc.tile_pool(name="prow", bufs=1) as prow,
            tc.tile_pool(name="pexp", bufs=2) as pexp,
            tc.tile_pool(name="pout", bufs=2) as pout,
            tc.tile_pool(name="pp", bufs=3, space="PSUM") as pp,
        ):
            import contextlib
            loop_ctx = tc.For_i(0, loop_n, 1) if loop_n else contextlib.nullcontext()
            # constants
            ones_col = pc.tile([128, 1], BF16, tag="onesc")
            nc.scalar.dma_start(out=ones_col, in_=onesc_d)
            ones_row = pc.tile([1, 128], F32R, tag="onesr")
            nc.scalar.dma_start(out=ones_row, in_=onesr_d)
            ident_sb = pc.tile([128, 128], F32R, tag="ident")
            nc.scalar.dma_start(out=ident_sb, in_=ident_d)
            ck_sb = pc.tile([128, 2, W], BF16, tag="c1")
            sk_sb = pc.tile([128, 2, W], BF16, tag="c2")
            skn_sb = pc.tile([128, 2, W], BF16, tag="c3")
            cq_sb = pc.tile([128, 2, CH], BF16, tag="c1")
            sq_sb = pc.tile([128, 2, CH], BF16, tag="c2")
            sqn_sb = pc.tile([128, 2, CH], BF16, tag="c3")
            w1p_sb = pc.tile([128, 4], F32, tag="w1p")
            nc.scalar.dma_start(out=w1p_sb, in_=w1p)
            rsk_sb = pc.tile([128, 4 * NKV], F32, tag="rsk")

            def rsqrt_sb(out_sb, t_sb, nfree, scale=1.0):
                """out = scale * t^-0.5 for SBUF f32 t (t in ~[0.3, 3.2]):
                cubic Horner seed + Newton iter in bf16, then one fp32
                Newton iter. y^2 goes through ACT Square."""
                z = ptmp.tile([128, nfree], BF16, tag="nwA", bufs=1)
                nc.vector.tensor_scalar(z, t_sb, RSQ_P3, RSQ_P2,
                                        op0=OP.mult, op1=OP.add)
                z2 = ptmp.tile([128, nfree], BF16, tag="nwB", bufs=1)
                nc.vector.scalar_tensor_tensor(z2, z, 0.0, t_sb,
                                               op0=OP.add, op1=OP.mult)
                z3 = ptmp.tile([128, nfree], BF16, tag="nwA", bufs=1)
                nc.vector.scalar_tensor_tensor(z3, z2, RSQ_P1, t_sb,
                                               op0=OP.add, op1=OP.mult)
                y = ptmp.tile([128, nfree], BF16, tag="nwB", bufs=1)
                nc.vector.tensor_scalar(y, z3, 1.0, RSQ_P0,
                                        op0=OP.mult, op1=OP.add)
                for it in range(2):
                    dt_ = BF16 if it == 0 else F32
                    sqy = ptmp.tile([128, nfree], dt_, tag="nwA", bufs=1,
                                    name=f"sqy{it}")
                    nc.scalar.square(sqy, y)
                    u = ptmp.tile([128, nfree], dt_, tag="nwC", bufs=1,
                                  name=f"nwu{it}")
                    nc.vector.tensor_mul(u, sqy, t_sb)
                    v = ptmp.tile([128, nfree], dt_, tag="nwA", bufs=1,
                                  name=f"nwv{it}")
                    nc.vector.tensor_scalar(v, u, -0.5, 1.5,
                                            op0=OP.mult, op1=OP.add)
                    if it == 0:
                        y1 = ptmp.tile([128, nfree], BF16, tag="nwD", bufs=1)
                        nc.vector.tensor_mul(y1, y, v)
                        y = y1
                    else:
                        nc.vector.scalar_tensor_tensor(out_sb, v, scale, y,
                                                       op0=OP.mult,
                                                       op1=OP.mult)

            def wrope_pair(ps0p, ps1p, hat, base, wcol0, wcol1, cos2, sin2,
                           nsin2, nfree):
                """rope((1+w) * raw) for a HEAD PAIR. ps0p/ps1p: PSUM
                [128, 2, nfree] (dim 1 = head in pair, halves split across
                the two tiles). Writes hat slots {base, base+2} (first
                halves) and {base+1, base+3} (second halves), bf16."""
                h0 = hat[:, base:base + 4:2, :]
                h1 = hat[:, base + 1:base + 4:2, :]
                # both ps0p reads first, then ps1p, so the PSUM banks free
                # as early as possible for the next consumer
                a = ptmp.tile([128, 2, nfree], BF16, tag="ra", bufs=1)
                b2 = ptmp.tile([128, 2, nfree], BF16, tag="rb", bufs=1)
                nc.vector.scalar_tensor_tensor(a, ps0p, wcol0, cos2,
                                               op0=OP.mult, op1=OP.mult)
                nc.vector.scalar_tensor_tensor(b2, ps0p, wcol0, sin2,
                                               op0=OP.mult, op1=OP.mult)
                bn = ptmp.tile([128, 2, nfree], BF16, tag="nwA", bufs=1)
                a2 = ptmp.tile([128, 2, nfree], BF16, tag="nwB", bufs=1)
                nc.vector.scalar_tensor_tensor(bn, ps1p, wcol1, nsin2,
                                               op0=OP.mult, op1=OP.mult)
                nc.vector.scalar_tensor_tensor(a2, ps1p, wcol1, cos2,
                                               op0=OP.mult, op1=OP.mult)
                nc.vector.tensor_add(h0, a, bn)
                nc.vector.tensor_add(h1, a2, b2)

            with loop_ctx:
                khat = pkv.tile([128, 2 * NKV, W], BF16, tag="khat")
                v_sb = pkv.tile([128, 4, NKV * D], BF16, tag="v")
                qhat = pq.tile([128, 2 * NH, CH], BF16, tag="qhat")

                # ---- Phase 1: local kv projection, all 4 heads ----
                xkv_sb = px.tile([128, KT, W], BF16, tag="x")
                kw_sb = [pkw.tile([128, KT, 512], BF16, tag="w",
                                  name=f"kw{wv}") for wv in range(2)]
                CHUNKS = [(0, 1), (1, 2), (2, 4), (4, 8), (8, 14), (14, 20)]
                for lo, hi in CHUNKS:
                    sl = slice(lo, hi)
                    nc.sync.dma_start(out=kw_sb[0][:, sl, :], in_=kwa_d[:, sl, :])
                    nc.sync.dma_start(out=xkv_sb[:, sl, :], in_=xkv_d[:, sl, :])
                for c in range(NKC):
                    sl = slice(c * CKT, (c + 1) * CKT)
                    nc.sync.dma_start(out=kw_sb[1][:, sl, :], in_=kwb_d[:, sl, :])
                nc.scalar.dma_start(out=ck_sb, in_=ck)
                nc.scalar.dma_start(out=sk_sb, in_=sk)
                nc.scalar.dma_start(out=skn_sb, in_=skn)

                # k projection: 2 waves = 2 head pairs.  kps[m][:, gl, :] is
                # head gl's half m.  ssT accumulates transposed sums of
                # squares so rs_k becomes a per-partition tanh scale.
                ssT = pp.tile([128, 16], F32, tag="b1", name="ssT", bufs=2)
                for wv in range(2):
                    kps = [pp.tile([128, 2, W], F32, tag="b2",
                                   name=f"kps{wv}{m}") for m in range(2)]
                    for kt in range(KT):
                        for gl in range(2):
                            for m in range(2):
                                nc.tensor.matmul(
                                    kps[m][:, gl, :],
                                    kw_sb[wv][:, kt,
                                              gl * 256 + m * 128:
                                              gl * 256 + (m + 1) * 128],
                                    xkv_sb[:, kt, :],
                                    start=(kt == 0), stop=(kt == KT - 1))
                    for gl in range(2):
                        sqk = [ptmp.tile([128, W], BF16, tag="tA",
                                         name=f"sqk{wv}{gl}{m}")
                               for m in range(2)]
                        for m in range(2):
                            nc.scalar.square(sqk[m], kps[m][:, gl, :])
                        for mlk in range(4):
                            idx = (2 * wv + gl) * 4 + mlk
                            for m in range(2):
                                nc.tensor.matmul(
                                    ssT[:, idx:idx + 1],
                                    sqk[m][:, mlk * 128:(mlk + 1) * 128],
                                    ones_col,
                                    start=(m == 0), stop=(m == 1))
                    wrope_pair(kps[0], kps[1], khat, 4 * wv,
                               w1p_sb[:, 2:3], w1p_sb[:, 3:4],
                               ck_sb, sk_sb, skn_sb, W)
                # rsk = c0 * (ssT/D + EPS)^-0.5 for all 4 kv heads at once
                tk = ptmp.tile([128, 16], F32, tag="tq", bufs=1, name="tk")
                nc.vector.tensor_scalar(tk, ssT, 1.0 / D, EPS,
                                        op0=OP.mult, op1=OP.add)
                rsqrt_sb(rsk_sb, tk, 16, scale=C0)

                # v projection: 2 waves x 2 heads
                vw_sb = [pkw.tile([128, KT, 512], BF16, tag="w",
                                  name=f"vw{wv}") for wv in range(2)]
                for wv, vd in ((0, vwa_d), (1, vwb_d)):
                    for c in range(NKC):
                        sl = slice(c * CKT, (c + 1) * CKT)
                        nc.sync.dma_start(out=vw_sb[wv][:, sl, :],
                                          in_=vd[:, sl, :])
                for wv in range(2):
                    vps = [pp.tile([128, 2, 512], F32, tag="b2",
                                   name=f"vps{wv}{mm2}") for mm2 in range(2)]
                    for kt in range(KT):
                        for m in range(4):
                            nc.tensor.matmul(
                                vps[m // 2][:, m % 2, :],
                                xkv_sb[:, kt, m * 128:(m + 1) * 128],
                                vw_sb[wv][:, kt, :],
                                start=(kt == 0), stop=(kt == KT - 1))
                    for mm2 in range(2):
                        # rows (2*mm2, 2*mm2+1) of the window chunk dim
                        nc.vector.tensor_copy(
                            v_sb[:, 2 * mm2:2 * mm2 + 2,
                                 wv * 512:(wv + 1) * 512],
                            vps[mm2])

                nc.sync.dma_start(out=cq_sb, in_=cq)
                nc.sync.dma_start(out=sq_sb, in_=sq)
                nc.sync.dma_start(out=sqn_sb, in_=sqn)

                # ---- Phase 2+3: pair pipeline  qproj(p) | scores(p-2) |
                #      attn_out(p-3) ----
                xq_sb = px.tile([128, KT, CH], BF16, tag="x")
                for c in range(NKC):
                    sl = slice(c * CKT, (c + 1) * CKT)
                    nc.sync.dma_start(out=xq_sb[:, sl, :], in_=xq_d[:, sl, :])
                aoT = px.tile([128, 2 * NH, CH], BF16, tag="x")

                qps_live = {}      # p -> [2 PSUM [128,2,CH] tiles (halves)]
                sqt_live = {}      # p -> [2 bf16 [128,2,CH] sq tiles]
                ssq_live = {}      # p -> transposed sums of squares [128,8] PSUM
                rsq8_live = {}     # p -> rs_q compact [128,8] f32
                rbq_live = {}      # p -> rs_q broadcast [128,2,CH] f32 PSUM
                exps_live = {}     # p -> exp tile [128, 4, 2, CH]
                dnrow_live = {}    # p -> [1, 2, CH] f32r
                qn_live = {}       # p -> normalized qhat [128, 4, CH] bf16
                qw_tiles = {}

                def qw_prefetch(p):
                    qw_t = pkw.tile([128, KT, 2 * D], BF16, tag="w",
                                    name=f"qwp{p}")
                    nc.sync.dma_start(out=qw_t, in_=qw_d[p])
                    qw_tiles[p] = qw_t

                def qproj_mms(p):
                    qw_t = qw_tiles.pop(p)
                    qps = [pp.tile([128, 2, CH], F32, tag="b2",
                                   name=f"qps{p}{m}") for m in range(2)]
                    for kt in range(KT):
                        for i in range(2):
                            for m in range(2):
                                nc.tensor.matmul(
                                    qps[m][:, i, :],
                                    qw_t[:, kt,
                                         i * 256 + m * 128:
                                         i * 256 + (m + 1) * 128],
                                    xq_sb[:, kt, :],
                                    start=(kt == 0), stop=(kt == KT - 1))
                    sqt = [ptmp.tile([128, 2, CH], BF16, tag="tA",
                                     name=f"sqt{p}{m}") for m in range(2)]
                    for m in range(2):
                        nc.scalar.square(sqt[m], qps[m])
                    qps_live[p] = qps
                    sqt_live[p] = sqt

                def rope_chain(p):
                    qps = qps_live.pop(p)
                    wrope_pair(qps[0], qps[1], qhat, 4 * p,
                               w1p_sb[:, 0:1], w1p_sb[:, 1:2],
                               cq_sb, sq_sb, sqn_sb, CH)

                def ssq_mms(p):
                    """Transposed sums of squares: [128(row), 8] where
                    col idx = i*4 + ch covers (head-in-pair i, 128-row
                    chunk ch)."""
                    sqt = sqt_live.pop(p)
                    ssq = pp.tile([128, 8], F32, tag="b1", bufs=2,
                                  name=f"ssq{p}")
                    for i in range(2):
                        for ch in range(4):
                            idx = i * 4 + ch
                            for m in range(2):
                                nc.tensor.matmul(
                                    ssq[:, idx:idx + 1],
                                    sqt[m][:, i, ch * 128:(ch + 1) * 128],
                                    ones_col,
                                    start=(m == 0), stop=(m == 1))
                    ssq_live[p] = ssq

                def newton_c(p):
                    """rs_q on the compact [128,8] layout (cheap on DVE)."""
                    ssq = ssq_live.pop(p)
                    t8 = ptmp.tile([128, 8], F32, tag="tq", bufs=1,
                                   name=f"t8{p}")
                    nc.vector.tensor_scalar(t8, ssq, 1.0 / D, EPS,
                                            op0=OP.mult, op1=OP.add)
                    rsq8 = ptmp.tile([128, 8], F32R, tag="rbB", bufs=2,
                                     name=f"rsq8{p}")
                    rsqrt_sb(rsq8, t8, 8)
                    rsq8_live[p] = rsq8

                def trans_bcast(p):
                    """Broadcast compact rs_q to [128, 2, CH] PSUM via
                    stride-0 lhsT x identity: out[d, i, ch*128+j] =
                    rsq8[j, i*4+ch] for every partition d."""
                    rsq8r = rsq8_live.pop(p)
                    rbq_ps = pp.tile([128, 2, CH], F32, tag="b2",
                                     name=f"rbq{p}")
                    for i in range(2):
                        for ch in range(4):
                            idx = i * 4 + ch
                            nc.tensor.matmul(
                                rbq_ps[:, i, ch * 128:(ch + 1) * 128],
                                rsq8r[:, idx:idx + 1].to_broadcast([128, 128]),
                                ident_sb,
                                start=True, stop=True)
                    rbq_live[p] = rbq_ps

                def qnorm_mul(p):
                    rbq = rbq_live.pop(p)
                    qn = pq.tile([128, 4, CH], BF16, tag="qhatn", bufs=2,
                                 name=f"qhatn{p}")
                    for i in range(2):
                        for dk in range(2):
                            nc.vector.tensor_mul(
                                qn[:, 2 * i + dk, :],
                                qhat[:, 4 * p + 2 * i + dk, :],
                                rbq[:, i, :])
                    qn_live[p] = qn

                def sps_softmax(p):
                    g = p
                    qn = qn_live.pop(p)
                    exps = pexp.tile([128, 4, 2, CH], BF16, tag="exps",
                                     name=f"exps{p}")
                    for mlk in range(4):
                        sps = pp.tile([128, 2, CH], F32, tag="b2",
                                      name=f"sps{p}{mlk}")
                        for i in range(2):
                            for dk in range(2):
                                nc.tensor.matmul(
                                    sps[:, i, :],
                                    khat[:, 2 * g + dk,
                                         mlk * 128:(mlk + 1) * 128],
                                    qn[:, 2 * i + dk, :],
                                    start=(dk == 0), stop=(dk == 1))
                        nc.scalar.activation(
                            sps, sps, AF.Tanh,
                            scale=rsk_sb[:, g * 4 + mlk:g * 4 + mlk + 1])
                        nc.scalar.activation(exps[:, mlk, :, :], sps, AF.Exp,
                                             scale=SOFTCAP)
                    exps_live[p] = exps

                def dn_part(p):
                    exps = exps_live[p]
                    dn_ps = pp.tile([1, 2, CH], F32, tag="b2",
                                    name=f"dn{p}")
                    for i in range(2):
                        for mlk in range(4):
                            nc.tensor.matmul(dn_ps[:, i, :], ones_col,
                                             exps[:, mlk, i, :],
                                             start=(mlk == 0),
                                             stop=(mlk == 3))
                    dnrow = prow.tile([1, 2, CH], F32R, tag="drow",
                                      name=f"dnrow{p}")
                    nc.scalar.copy(dnrow, dn_ps)
                    dnrow_live[p] = dnrow

                def ops_fin(p):
                    g = p
                    exps = exps_live.pop(p)
                    opst = []
                    for dh in range(2):
                        ops = pp.tile([128, 2, CH], F32, tag="b2",
                                      name=f"ops{p}{dh}")
                        for i in range(2):
                            for klk in range(4):
                                nc.tensor.matmul(
                                    ops[:, i, :],
                                    v_sb[:, klk,
                                         g * 256 + dh * 128:
                                         g * 256 + dh * 128 + 128],
                                    exps[:, klk, i, :],
                                    start=(klk == 0), stop=(klk == 3))
                        opst.append(ops)
                    rbat_ps = pp.tile([128, 2, CH], F32, tag="b2",
                                      name=f"rbat{p}")
                    dnrow = dnrow_live.pop(p)
                    for i in range(2):
                        nc.tensor.matmul(rbat_ps[:, i, :], ones_row,
                                         dnrow[:, i, :], start=True,
                                         stop=True)
                    rbat = ptmp.tile([128, 2, CH], F32, tag="rbC",
                                     name=f"rbat{p}")
                    nc.vector.reciprocal_approx_fast(rbat, rbat_ps)
                    for dh in range(2):
                        # aoT slots {4p+dh, 4p+2+dh}
                        nc.vector.tensor_mul(
                            aoT[:, 4 * p + dh:4 * p + dh + 3:2, :],
                            opst[dh], rbat)

                qw_prefetch(0)
                for s in range(NP + 3):
                    if 0 <= s - 3 < NP:
                        dn_part(s - 3)
                    if 0 <= s - 1 < NP:
                        newton_c(s - 1)
                    if 0 <= s - 3 < NP:
                        ops_fin(s - 3)
                    if 0 <= s - 1 < NP:
                        trans_bcast(s - 1)
                        qnorm_mul(s - 1)
                    if s + 1 < NP:
                        qw_prefetch(s + 1)
                    if s < NP:
                        qproj_mms(s)
                    if s < NP:
                        ssq_mms(s)
                    if s < NP:
                        rope_chain(s)
                    if 0 <= s - 2 < NP:
                        sps_softmax(s - 2)

                # ---- Phase 4: o projection (outputs transposed: yT) ----
                for mp2 in range(HID // 256):
                    yps = pp.tile([128, 2, CH], F32, tag="b2",
                                  name=f"yps{mp2}")
                    for j in range(2):
                        mp = 2 * mp2 + j
                        owc = pow_.tile([128, 16, 128], BF16, tag="ow",
                                        name=f"ow{mp}")
                        nc.sync.dma_start(out=owc, in_=ow_d[:, mp, :, :])
                        for kk in range(16):
                            nc.tensor.matmul(yps[:, j, :], owc[:, kk, :],
                                             aoT[:, kk, :],
                                             start=(kk == 0), stop=(kk == 15))
                    yst = pout.tile([128, 2, CH], F32, tag="yst")
                    nc.scalar.copy(yst, yps)
                    for j in range(2):
                        nc.sync.dma_start(
                            out=yT[(2 * mp2 + j) * 128:
                                   (2 * mp2 + j + 1) * 128, :],
                            in_=yst[:, j, :])

    nc.compile()

    return nc


_NC_CACHE = {}


def _get_nc():
    if "nc" not in _NC_CACHE:
        _NC_CACHE["nc"] = _build()
    return _NC_CACHE["nc"]


def _rope_tables():
    inv_freq = 1.0 / (ROPE_BASE ** (np.arange(0, D, 2, dtype=np.float32) / D))
    t = np.arange(L, dtype=np.float32)
    freqs = np.outer(t, inv_freq)                     # (L, 128)
    return (np.ascontiguousarray(np.cos(freqs).T.astype(np.float32)),
            np.ascontiguousarray(np.sin(freqs).T.astype(np.float32)))


def _part_major(mat_t, free):
    """(HID_like, free) feature-major -> (128, KT_like, free) partition-major
    bf16 blocks: out[p, kt, f] = mat_t[kt*128 + p, f]."""
    r = mat_t.shape[0]
    return np.ascontiguousarray(
        mat_t.reshape(r // 128, 128, free).transpose(1, 0, 2).astype(NPBF16))


def _dup2(tab):
    """(128, N) f32 -> (128, 2, N) bf16 duplicated over dim 1."""
    return np.ascontiguousarray(
        np.repeat(tab[:, None, :], 2, axis=1).astype(NPBF16))


def _prep_in_maps(x, q_w, k_w, v_w, o_w, q_norm_w, k_norm_w):
    # q_w per pair: (128, KT, 512); feats = head_in_pair*256 + d
    qw_p = np.ascontiguousarray(
        q_w.reshape(NP, 2 * D, KT, 128).transpose(0, 3, 2, 1).astype(NPBF16))
    kwT = np.ascontiguousarray(k_w.T)                 # (HID, 1024)
    vwT = np.ascontiguousarray(v_w.T)
    kw_a = _part_major(kwT[:, :512], 512)
    kw_b = _part_major(kwT[:, 512:], 512)
    vw_a = _part_major(vwT[:, :512], 512)
    vw_b = _part_major(vwT[:, 512:], 512)
    # o_w: (128, 20, 16, 128); ow_p[p, mp, kk, f] = o_w[mp*128+f, kk*128+p]
    ow_p = np.ascontiguousarray(
        o_w.reshape(HID // 128, 128, 16, 128).transpose(3, 0, 2, 1)
        .astype(NPBF16))
    cosT, sinT = _rope_tables()                        # (128, L) each
    w1p = np.empty((128, 4), dtype=np.float32)
    w1p[:, 0] = 1.0 + q_norm_w[:128]
    w1p[:, 1] = 1.0 + q_norm_w[128:]
    w1p[:, 2] = 1.0 + k_norm_w[:128]
    w1p[:, 3] = 1.0 + k_norm_w[128:]

    kv_lo = L - W
    xkv_b = [_part_major(np.ascontiguousarray(x[b, kv_lo:, :].T), W)
             for b in range(B)]
    ck_t = _dup2(cosT[:, kv_lo:])
    sk_t = _dup2(sinT[:, kv_lo:])
    skn_t = _dup2(-sinT[:, kv_lo:])

    in_maps = []
    for c in range(NCORES):
        b, j = divmod(c, 4)
        rows = slice(j * CH, (j + 1) * CH)
        in_maps.append({
            "xq": _part_major(np.ascontiguousarray(x[b, rows, :].T), CH),
            "xkv": xkv_b[b],
            "qw": qw_p, "kwa": kw_a, "kwb": kw_b,
            "vwa": vw_a, "vwb": vw_b, "ow": ow_p,
            "cq": _dup2(cosT[:, rows]),
            "sq": _dup2(sinT[:, rows]),
            "sqn": _dup2(-sinT[:, rows]),
            "ck": ck_t, "sk": sk_t, "skn": skn_t,
            "w1p": w1p,
            "onesc": np.ones((128, 1), dtype=NPBF16),
            "onesr": np.ones((1, 128), dtype=np.float32),
            "ident": np.eye(128, dtype=np.float32),
        })
    return in_maps


def kernel(x, mask, q_w, k_w, v_w, o_w, q_norm_w, k_norm_w):
    x = np.asarray(x, dtype=np.float32)
    q_w = np.asarray(q_w, dtype=np.float32)
    k_w = np.asarray(k_w, dtype=np.float32)
    v_w = np.asarray(v_w, dtype=np.float32)
    o_w = np.asarray(o_w, dtype=np.float32)
    q_norm_w = np.asarray(q_norm_w, dtype=np.float32)
    k_norm_w = np.asarray(k_norm_w, dtype=np.float32)

    nc = _get_nc()
    in_maps = _prep_in_maps(x, q_w, k_w, v_w, o_w, q_norm_w, k_norm_w)

    res = run_bass_kernel_spmd(nc, in_maps, list(range(NCORES)))
    _NC_CACHE["last_res"] = res

    out = np.empty((B, L, HID), dtype=np.float32)
    for c in range(NCORES):
        b, j = divmod(c, 4)
        out[b, j * CH:(j + 1) * CH, :] = res.results[c]["yT"].T
    return out



# revision 24
# speedup vs baseline: 1.0493x; 1.0148x over previous
"""Gemma3 sliding-window attention kernel for 8 Trainium2 NeuronCores.

Sharding: core c handles batch b = c//4, query-row chunk j = c%4 (512 rows).
The reference keeps only the LAST 512 key columns for every query row, so
each core computes k/v projections for rows 1536:2048 of its batch — all 4
kv heads locally (no collectives; the duplicated kv compute is cheaper than
the AllGather latency on hardware).

All matmul operands stream from HBM in bf16 (cast host-side); PSUM
accumulation is fp32 and softmax math stays fp32.

The attention works in HEAD PAIRS (each q-head pair shares one kv head, so
scores/softmax operands and the tanh scale coincide): DVE/ACT chain ops run
on [128, 2*512] pair tiles, halving the per-op overhead that dominates on
hardware. RMS normalization is deferred off the critical path: khat/qhat
hold rope((1+w)*raw); rs_k folds into the softcap tanh's per-partition
scale AP; rs_q is applied to qhat one pipeline step later from a
PE-broadcast row of sums of squares, with rsqrt done as a cubic seed + two
Newton steps (bf16 then fp32). Pipeline: step s issues qproj(pair s),
scores(s-2), attn_out(s-3).
"""

import numpy as np
import ml_dtypes

import concourse.bacc as bacc
import concourse.tile as tile
from concourse import mybir
from concourse.bass_utils import run_bass_kernel_spmd

F32 = mybir.dt.float32
F32R = mybir.dt.float32r
BF16 = mybir.dt.bfloat16
AF = mybir.ActivationFunctionType
OP = mybir.AluOpType

B, L, HID = 2, 2048, 2560
NH, NKV, D = 8, 4, 256
NP = NH // 2       # head pairs; pair p = heads (2p, 2p+1), kv head p
W = 512            # effective kv window (last W positions of the sequence)
CH = 512           # query rows per core
NCORES = 8
KT = HID // 128    # 20 contraction tiles for the projections
EPS = 1e-6
SOFTCAP = 50.0
SCALE = D ** -0.5
ROPE_BASE = 10000.0
NPBF16 = ml_dtypes.bfloat16
# tanh input scale c folded into the rs_k rsqrt: tanh((c*rs_k) * (rs_q*s))
C0 = SCALE / SOFTCAP
# cubic minimax seed for rsqrt on t in [0.3, 3.2] (rel err 4.9%), then two
# Newton steps (first bf16, second fp32) -> ~1e-4
RSQ_P3, RSQ_P2, RSQ_P1, RSQ_P0 = (-0.11751866, 0.81282722,
                                  -1.93345784, 2.24612936)


def _build(loop_n=None):
    nc = bacc.Bacc("TRN2", target_bir_lowering=False, debug=False,
                   num_devices=NCORES)
    xq_d = nc.dram_tensor("xq", [128, KT, CH], BF16, kind="ExternalInput").ap()
    xkv_d = nc.dram_tensor("xkv", [128, KT, W], BF16, kind="ExternalInput").ap()
    qw_d = nc.dram_tensor("qw", [NP, 128, KT, 2 * D], BF16,
                          kind="ExternalInput").ap()
    kwa_d = nc.dram_tensor("kwa", [128, KT, 512], BF16, kind="ExternalInput").ap()
    kwb_d = nc.dram_tensor("kwb", [128, KT, 512], BF16, kind="ExternalInput").ap()
    vwa_d = nc.dram_tensor("vwa", [128, KT, 512], BF16, kind="ExternalInput").ap()
    vwb_d = nc.dram_tensor("vwb", [128, KT, 512], BF16, kind="ExternalInput").ap()
    ow_d = nc.dram_tensor("ow", [128, HID // 128, 16, 128], BF16,
                          kind="ExternalInput").ap()
    # rope tables duplicated over the pair dim for [128, 2, *] chain ops
    cq = nc.dram_tensor("cq", [128, 2, CH], BF16, kind="ExternalInput").ap()
    sq = nc.dram_tensor("sq", [128, 2, CH], BF16, kind="ExternalInput").ap()
    sqn = nc.dram_tensor("sqn", [128, 2, CH], BF16, kind="ExternalInput").ap()
    ck = nc.dram_tensor("ck", [128, 2, W], BF16, kind="ExternalInput").ap()
    sk = nc.dram_tensor("sk", [128, 2, W], BF16, kind="ExternalInput").ap()
    skn = nc.dram_tensor("skn", [128, 2, W], BF16, kind="ExternalInput").ap()
    # columns: 1+qnw[:128], 1+qnw[128:], 1+knw[:128], 1+knw[128:]
    w1p = nc.dram_tensor("w1p", [128, 4], F32, kind="ExternalInput").ap()
    # (1+w)^-2 correction columns so sums of squares of the (1+w)-scaled
    # copies recover the raw-q/k norms; same column order as w1p
    ccol_d = nc.dram_tensor("ccol", [128, 4], BF16, kind="ExternalInput").ap()
    onesc_d = nc.dram_tensor("onesc", [128, 1], BF16, kind="ExternalInput").ap()
    onesr_d = nc.dram_tensor("onesr", [1, 128], F32R, kind="ExternalInput").ap()
    ident_d = nc.dram_tensor("ident", [128, 128], F32R, kind="ExternalInput").ap()
    yT = nc.dram_tensor("yT", [HID, CH], F32, kind="ExternalOutput").ap()

    NKC = 4
    CKT = KT // NKC

    with tile.TileContext(nc) as tc, \
            nc.allow_low_precision(reason='bf16 matmul operands'):
        with (
            tc.tile_pool(name="const", bufs=1) as pc,
            tc.tile_pool(name="px", bufs=2) as px,
            tc.tile_pool(name="pkw", bufs=2) as pkw,
            tc.tile_pool(name="pow", bufs=3) as pow_,
            tc.tile_pool(name="pkv", bufs=1) as pkv,
            tc.tile_pool(name="pq", bufs=1) as pq,
            tc.tile_pool(name="ptmp", bufs=2) as ptmp,
            tc.tile_pool(name="prow", bufs=1) as prow,
            tc.tile_pool(name="pexp", bufs=2) as pexp,
            tc.tile_pool(name="pout", bufs=2) as pout,
            tc.tile_pool(name="pp", bufs=3, space="PSUM") as pp,
        ):
            import contextlib
            loop_ctx = tc.For_i(0, loop_n, 1) if loop_n else contextlib.nullcontext()
            # constants
            ones_col = pc.tile([128, 1], BF16, tag="onesc")
            nc.scalar.dma_start(out=ones_col, in_=onesc_d)
            ones_row = pc.tile([1, 128], F32R, tag="onesr")
            nc.scalar.dma_start(out=ones_row, in_=onesr_d)
            ident_sb = pc.tile([128, 128], F32R, tag="ident")
            nc.scalar.dma_start(out=ident_sb, in_=ident_d)
            ck_sb = pc.tile([128, 2, W], BF16, tag="c1")
            sk_sb = pc.tile([128, 2, W], BF16, tag="c2")
            skn_sb = pc.tile([128, 2, W], BF16, tag="c3")
            cq_sb = pc.tile([128, 2, CH], BF16, tag="c1")
            sq_sb = pc.tile([128, 2, CH], BF16, tag="c2")
            sqn_sb = pc.tile([128, 2, CH], BF16, tag="c3")
            w1p_sb = pc.tile([128, 4], F32, tag="w1p")
            nc.scalar.dma_start(out=w1p_sb, in_=w1p)
            ccol_sb = pc.tile([128, 4], BF16, tag="ccol")
            nc.scalar.dma_start(out=ccol_sb, in_=ccol_d)
            rsk_sb = pc.tile([128, 4 * NKV], F32, tag="rsk")
            # warm the ACT table set (exp/tanh/square/copy) while the first
            # weight DMAs are still in flight
            warm = pc.tile([128, 1], BF16, tag="warm")
            nc.scalar.activation(warm, ones_col, AF.Tanh)

            def rsqrt_sb(out_sb, t_sb, nfree, scale=1.0):
                """out = scale * t^-0.5 for SBUF f32 t (t in ~[0.3, 3.2]):
                cubic Horner seed + Newton iter in bf16, then one fp32
                Newton iter. y^2 goes through ACT Square."""
                z = ptmp.tile([128, nfree], BF16, tag="nwA", bufs=1)
                nc.vector.tensor_scalar(z, t_sb, RSQ_P3, RSQ_P2,
                                        op0=OP.mult, op1=OP.add)
                z2 = ptmp.tile([128, nfree], BF16, tag="nwB", bufs=1)
                nc.vector.scalar_tensor_tensor(z2, z, 0.0, t_sb,
                                               op0=OP.add, op1=OP.mult)
                z3 = ptmp.tile([128, nfree], BF16, tag="nwA", bufs=1)
                nc.vector.scalar_tensor_tensor(z3, z2, RSQ_P1, t_sb,
                                               op0=OP.add, op1=OP.mult)
                y = ptmp.tile([128, nfree], BF16, tag="nwB", bufs=1)
                nc.vector.tensor_scalar(y, z3, 1.0, RSQ_P0,
                                        op0=OP.mult, op1=OP.add)
                for it in range(2):
                    dt_ = BF16 if it == 0 else F32
                    sqy = ptmp.tile([128, nfree], dt_, tag="nwA", bufs=1,
                                    name=f"sqy{it}")
                    nc.vector.tensor_mul(sqy, y, y)
                    u = ptmp.tile([128, nfree], dt_, tag="nwC", bufs=1,
                                  name=f"nwu{it}")
                    nc.vector.tensor_mul(u, sqy, t_sb)
                    v = ptmp.tile([128, nfree], dt_, tag="nwA", bufs=1,
                                  name=f"nwv{it}")
                    nc.vector.tensor_scalar(v, u, -0.5, 1.5,
                                            op0=OP.mult, op1=OP.add)
                    if it == 0:
                        y1 = ptmp.tile([128, nfree], BF16, tag="nwD", bufs=1)
                        nc.vector.tensor_mul(y1, y, v)
                        y = y1
                    else:
                        nc.vector.scalar_tensor_tensor(out_sb, v, scale, y,
                                                       op0=OP.mult,
                                                       op1=OP.mult)

            def psum_evac(ps0p, ps1p, wcol0, wcol1, nfree, nm):
                """Evacuate the projection PSUM pair through ACT copies that
                fold in the (1+w) scale; bf16 outputs let the rope chain run
                at the 2x DVE rate and free the PSUM banks after two ops."""
                c0 = ptmp.tile([128, 2, nfree], BF16, tag="qc0", bufs=2,
                               name=f"c0{nm}")
                c1 = ptmp.tile([128, 2, nfree], BF16, tag="qc1", bufs=2,
                               name=f"c1{nm}")
                nc.scalar.activation(c0, ps0p, AF.Copy, scale=wcol0)
                nc.scalar.activation(c1, ps1p, AF.Copy, scale=wcol1)
                return c0, c1

            def wrope_pair(c0, c1, hat, base, cos2, sin2, nsin2, nfree):
                """rope for a HEAD PAIR from the bf16 copies. Writes hat
                slots {base, base+2} (first halves) and {base+1, base+3}
                (second halves), bf16."""
                h0 = hat[:, base:base + 4:2, :]
                h1 = hat[:, base + 1:base + 4:2, :]
                a = ptmp.tile([128, 2, nfree], BF16, tag="ra", bufs=1)
                b2 = ptmp.tile([128, 2, nfree], BF16, tag="rb", bufs=1)
                nc.vector.tensor_mul(a, c0, cos2)
                nc.vector.tensor_mul(b2, c0, sin2)
                bn = ptmp.tile([128, 2, nfree], BF16, tag="nwA", bufs=1)
                a2 = ptmp.tile([128, 2, nfree], BF16, tag="nwB", bufs=1)
                nc.vector.tensor_mul(bn, c1, nsin2)
                nc.vector.tensor_mul(a2, c1, cos2)
                nc.vector.tensor_add(h0, a, bn)
                nc.vector.tensor_add(h1, a2, b2)

            with loop_ctx:
                khat = pkv.tile([128, 2 * NKV, W], BF16, tag="khat")
                v_sb = pkv.tile([128, 4, NKV * D], BF16, tag="v")
                qhat = pq.tile([128, 2 * NH, CH], BF16, tag="qhat")

                # ---- Phase 1: local kv projection, all 4 heads ----
                xkv_sb = px.tile([128, KT, W], BF16, tag="x")
                kw_sb = [pkw.tile([128, KT, 512], BF16, tag="w",
                                  name=f"kw{wv}") for wv in range(2)]
                CHUNKS = [(0, 1), (1, 2), (2, 4), (4, 8), (8, 14), (14, 20)]
                for lo, hi in CHUNKS:
                    sl = slice(lo, hi)
                    nc.sync.dma_start(out=kw_sb[0][:, sl, :], in_=kwa_d[:, sl, :])
                    nc.sync.dma_start(out=xkv_sb[:, sl, :], in_=xkv_d[:, sl, :])
                for c in range(NKC):
                    sl = slice(c * CKT, (c + 1) * CKT)
                    nc.sync.dma_start(out=kw_sb[1][:, sl, :], in_=kwb_d[:, sl, :])
                nc.scalar.dma_start(out=ck_sb, in_=ck)
                nc.scalar.dma_start(out=sk_sb, in_=sk)
                nc.scalar.dma_start(out=skn_sb, in_=skn)

                # k projection: 2 waves = 2 head pairs.  kps[m][:, gl, :] is
                # head gl's half m.  ssT accumulates transposed sums of
                # squares so rs_k becomes a per-partition tanh scale.
                ssT = pp.tile([128, 16], F32, tag="b1", name="ssT", bufs=2)
                for wv in range(2):
                    kps = [pp.tile([128, 2, W], F32, tag="b2",
                                   name=f"kps{wv}{m}") for m in range(2)]
                    for kt in range(KT):
                        for gl in range(2):
                            for m in range(2):
                                nc.tensor.matmul(
                                    kps[m][:, gl, :],
                                    kw_sb[wv][:, kt,
                                              gl * 256 + m * 128:
                                              gl * 256 + (m + 1) * 128],
                                    xkv_sb[:, kt, :],
                                    start=(kt == 0), stop=(kt == KT - 1))
                    kc = psum_evac(kps[0], kps[1], w1p_sb[:, 2:3],
                                   w1p_sb[:, 3:4], W, f"k{wv}")
                    sqk = [ptmp.tile([128, 2, W], BF16, tag="tA",
                                     name=f"sqk{wv}{m}") for m in range(2)]
                    for m in range(2):
                        nc.scalar.activation(sqk[m], kc[m], AF.Square)
                    for gl in range(2):
                        for mlk in range(4):
                            idx = (2 * wv + gl) * 4 + mlk
                            for m in range(2):
                                nc.tensor.matmul(
                                    ssT[:, idx:idx + 1],
                                    sqk[m][:, gl, mlk * 128:(mlk + 1) * 128],
                                    ccol_sb[:, 2 + m:3 + m],
                                    start=(m == 0), stop=(m == 1))
                    wrope_pair(kc[0], kc[1], khat, 4 * wv,
                               ck_sb, sk_sb, skn_sb, W)
                # rsk = c0 * (ssT/D + EPS)^-0.5 for all 4 kv heads at once
                tk = ptmp.tile([128, 16], F32, tag="tq", bufs=1, name="tk")
                nc.vector.tensor_scalar(tk, ssT, 1.0 / D, EPS,
                                        op0=OP.mult, op1=OP.add)
                rsqrt_sb(rsk_sb, tk, 16, scale=C0)

                # v projection: 2 waves x 2 heads
                vw_sb = [pkw.tile([128, KT, 512], BF16, tag="w",
                                  name=f"vw{wv}") for wv in range(2)]
                for wv, vd in ((0, vwa_d), (1, vwb_d)):
                    for c in range(NKC):
                        sl = slice(c * CKT, (c + 1) * CKT)
                        nc.sync.dma_start(out=vw_sb[wv][:, sl, :],
                                          in_=vd[:, sl, :])
                for wv in range(2):
                    vps = [pp.tile([128, 2, 512], F32, tag="b2",
                                   name=f"vps{wv}{mm2}") for mm2 in range(2)]
                    for kt in range(KT):
                        for m in range(4):
                            nc.tensor.matmul(
                                vps[m // 2][:, m % 2, :],
                                xkv_sb[:, kt, m * 128:(m + 1) * 128],
                                vw_sb[wv][:, kt, :],
                                start=(kt == 0), stop=(kt == KT - 1))
                    for mm2 in range(2):
                        # rows (2*mm2, 2*mm2+1) of the window chunk dim
                        nc.vector.tensor_copy(
                            v_sb[:, 2 * mm2:2 * mm2 + 2,
                                 wv * 512:(wv + 1) * 512],
                            vps[mm2])

                nc.sync.dma_start(out=cq_sb, in_=cq)
                nc.sync.dma_start(out=sq_sb, in_=sq)
                nc.sync.dma_start(out=sqn_sb, in_=sqn)

                # ---- Phase 2+3: pair pipeline  qproj(p) | scores(p-2) |
                #      attn_out(p-3) ----
                xq_sb = px.tile([128, KT, CH], BF16, tag="x")
                for c in range(NKC):
                    sl = slice(c * CKT, (c + 1) * CKT)
                    nc.sync.dma_start(out=xq_sb[:, sl, :], in_=xq_d[:, sl, :])
                aoT = px.tile([128, 2 * NH, CH], BF16, tag="x")

                qc_live = {}       # p -> [2 bf16 [128,2,CH] (1+w)-scaled copies]
                sqt_live = {}      # p -> [2 bf16 [128,2,CH] sq tiles]
                ssq_live = {}      # p -> transposed sums of squares [128,8] PSUM
                rsq8_live = {}     # p -> rs_q compact [128,8] f32
                rbq_live = {}      # p -> rs_q broadcast [128,2,CH] f32 PSUM
                exps_live = {}     # p -> exp tile [128, 4, 2, CH]
                dnrow_live = {}    # p -> [1, 2, CH] f32r
                qn_live = {}       # p -> normalized qhat [128, 4, CH] bf16
                qw_tiles = {}

                def qw_prefetch(p):
                    qw_t = pkw.tile([128, KT, 2 * D], BF16, tag="w",
                                    name=f"qwp{p}")
                    nc.sync.dma_start(out=qw_t, in_=qw_d[p])
                    qw_tiles[p] = qw_t

                def qproj_mms(p):
                    qw_t = qw_tiles.pop(p)
                    qps = [pp.tile([128, 2, CH], F32, tag="b2",
                                   name=f"qps{p}{m}") for m in range(2)]
                    for kt in range(KT):
                        for i in range(2):
                            for m in range(2):
                                nc.tensor.matmul(
                                    qps[m][:, i, :],
                                    qw_t[:, kt,
                                         i * 256 + m * 128:
                                         i * 256 + (m + 1) * 128],
                                    xq_sb[:, kt, :],
                                    start=(kt == 0), stop=(kt == KT - 1))
                    qc = psum_evac(qps[0], qps[1], w1p_sb[:, 0:1],
                                   w1p_sb[:, 1:2], CH, f"q{p}")
                    sqt = [ptmp.tile([128, 2, CH], BF16, tag="tA",
                                     name=f"sqt{p}{m}") for m in range(2)]
                    for m in range(2):
                        nc.scalar.activation(sqt[m], qc[m], AF.Square)
                    qc_live[p] = qc
                    sqt_live[p] = sqt

                def rope_chain(p):
                    qc = qc_live.pop(p)
                    wrope_pair(qc[0], qc[1], qhat, 4 * p,
                               cq_sb, sq_sb, sqn_sb, CH)

                def ssq_mms(p):
                    """Transposed sums of squares: [128(row), 8] where
                    col idx = i*4 + ch covers (head-in-pair i, 128-row
                    chunk ch)."""
                    sqt = sqt_live.pop(p)
                    ssq = pp.tile([128, 8], F32, tag="b1", bufs=2,
                                  name=f"ssq{p}")
                    for i in range(2):
                        for ch in range(4):
                            idx = i * 4 + ch
                            for m in range(2):
                                nc.tensor.matmul(
                                    ssq[:, idx:idx + 1],
                                    sqt[m][:, i, ch * 128:(ch + 1) * 128],
                                    ccol_sb[:, m:m + 1],
                                    start=(m == 0), stop=(m == 1))
                    ssq_live[p] = ssq

                def newton_c(p):
                    """rs_q on the compact [128,8] layout (cheap on DVE)."""
                    ssq = ssq_live.pop(p)
                    t8 = ptmp.tile([128, 8], F32, tag="tq", bufs=1,
                                   name=f"t8{p}")
                    nc.vector.tensor_scalar(t8, ssq, 1.0 / D, EPS,
                                            op0=OP.mult, op1=OP.add)
                    rsq8 = ptmp.tile([128, 8], F32R, tag="rbB", bufs=2,
                                     name=f"rsq8{p}")
                    rsqrt_sb(rsq8, t8, 8)
                    rsq8_live[p] = rsq8

                def trans_bcast(p):
                    """Broadcast compact rs_q to [128, 2, CH] PSUM via
                    stride-0 lhsT x identity: out[d, i, ch*128+j] =
                    rsq8[j, i*4+ch] for every partition d."""
                    rsq8r = rsq8_live.pop(p)
                    rbq_ps = pp.tile([128, 2, CH], F32, tag="b2",
                                     name=f"rbq{p}")
                    for i in range(2):
                        for ch in range(4):
                            idx = i * 4 + ch
                            nc.tensor.matmul(
                                rbq_ps[:, i, ch * 128:(ch + 1) * 128],
                                rsq8r[:, idx:idx + 1].to_broadcast([128, 128]),
                                ident_sb,
                                start=True, stop=True)
                    rbq_live[p] = rbq_ps

                def qnorm_mul(p):
                    rbq = rbq_live.pop(p)
                    qn = pq.tile([128, 4, CH], BF16, tag="qhatn", bufs=2,
                                 name=f"qhatn{p}")
                    for i in range(2):
                        for dk in range(2):
                            nc.vector.tensor_mul(
                                qn[:, 2 * i + dk, :],
                                qhat[:, 4 * p + 2 * i + dk, :],
                                rbq[:, i, :])
                    qn_live[p] = qn

                def sps_softmax(p):
                    g = p
                    qn = qn_live.pop(p)
                    exps = pexp.tile([128, 4, 2, CH], BF16, tag="exps",
                                     name=f"exps{p}")
                    for mlk in range(4):
                        sps = pp.tile([128, 2, CH], F32, tag="b2",
                                      name=f"sps{p}{mlk}")
                        for i in range(2):
                            for dk in range(2):
                                nc.tensor.matmul(
                                    sps[:, i, :],
                                    khat[:, 2 * g + dk,
                                         mlk * 128:(mlk + 1) * 128],
                                    qn[:, 2 * i + dk, :],
                                    start=(dk == 0), stop=(dk == 1))
                        nc.scalar.activation(
                            sps, sps, AF.Tanh,
                            scale=rsk_sb[:, g * 4 + mlk:g * 4 + mlk + 1])
                        nc.scalar.activation(exps[:, mlk, :, :], sps, AF.Exp,
                                             scale=SOFTCAP)
                    exps_live[p] = exps

                def dn_part(p):
                    exps = exps_live[p]
                    dn_ps = pp.tile([1, 2, CH], F32, tag="b2",
                                    name=f"dn{p}")
                    for i in range(2):
                        for mlk in range(4):
                            nc.tensor.matmul(dn_ps[:, i, :], ones_col,
                                             exps[:, mlk, i, :],
                                             start=(mlk == 0),
                                             stop=(mlk == 3))
                    dnrow = prow.tile([1, 2, CH], F32R, tag="drow",
                                      name=f"dnrow{p}")
                    nc.scalar.copy(dnrow, dn_ps)
                    dnrow_live[p] = dnrow

                def ops_fin(p):
                    g = p
                    exps = exps_live.pop(p)
                    opst = []
                    for dh in range(2):
                        ops = pp.tile([128, 2, CH], F32, tag="b2",
                                      name=f"ops{p}{dh}")
                        for i in range(2):
                            for klk in range(4):
                                nc.tensor.matmul(
                                    ops[:, i, :],
                                    v_sb[:, klk,
                                         g * 256 + dh * 128:
                                         g * 256 + dh * 128 + 128],
                                    exps[:, klk, i, :],
                                    start=(klk == 0), stop=(klk == 3))
                        opst.append(ops)
                    rbat_ps = pp.tile([128, 2, CH], F32, tag="b2",
                                      name=f"rbat{p}")
                    dnrow = dnrow_live.pop(p)
                    for i in range(2):
                        nc.tensor.matmul(rbat_ps[:, i, :], ones_row,
                                         dnrow[:, i, :], start=True,
                                         stop=True)
                    rbat = ptmp.tile([128, 2, CH], F32, tag="rbC",
                                     name=f"rbat{p}")
                    nc.vector.reciprocal_approx_fast(rbat, rbat_ps)
                    for dh in range(2):
                        # aoT slots {4p+dh, 4p+2+dh}
                        nc.vector.tensor_mul(
                            aoT[:, 4 * p + dh:4 * p + dh + 3:2, :],
                            opst[dh], rbat)

                qw_prefetch(0)
                for s in range(NP + 3):
                    if 0 <= s - 3 < NP:
                        dn_part(s - 3)
                    if 0 <= s - 1 < NP:
                        newton_c(s - 1)
                    if 0 <= s - 3 < NP:
                        ops_fin(s - 3)
                    if 0 <= s - 1 < NP:
                        trans_bcast(s - 1)
                        qnorm_mul(s - 1)
                    if 0 <= s - 2 < NP:
                        sps_softmax(s - 2)
                    if s + 1 < NP:
                        qw_prefetch(s + 1)
                    if s < NP:
                        qproj_mms(s)
                    if s < NP:
                        ssq_mms(s)
                    if s < NP:
                        rope_chain(s)

                # ---- Phase 4: o projection (outputs transposed: yT) ----
                for mp2 in range(HID // 256):
                    yps = pp.tile([128, 2, CH], F32, tag="b2",
                                  name=f"yps{mp2}")
                    for j in range(2):
                        mp = 2 * mp2 + j
                        owc = pow_.tile([128, 16, 128], BF16, tag="ow",
                                        name=f"ow{mp}")
                        nc.sync.dma_start(out=owc, in_=ow_d[:, mp, :, :])
                        for kk in range(16):
                            nc.tensor.matmul(yps[:, j, :], owc[:, kk, :],
                                             aoT[:, kk, :],
                                             start=(kk == 0), stop=(kk == 15))
                    yst = pout.tile([128, 2, CH], F32, tag="yst")
                    nc.scalar.copy(yst, yps)
                    for j in range(2):
                        nc.sync.dma_start(
                            out=yT[(2 * mp2 + j) * 128:
                                   (2 * mp2 + j + 1) * 128, :],
                            in_=yst[:, j, :])

    nc.compile()

    return nc


_NC_CACHE = {}


def _get_nc():
    if "nc" not in _NC_CACHE:
        _NC_CACHE["nc"] = _build()
    return _NC_CACHE["nc"]


def _rope_tables():
    inv_freq = 1.0 / (ROPE_BASE ** (np.arange(0, D, 2, dtype=np.float32) / D))
    t = np.arange(L, dtype=np.float32)
    freqs = np.outer(t, inv_freq)                     # (L, 128)
    return (np.ascontiguousarray(np.cos(freqs).T.astype(np.float32)),
            np.ascontiguousarray(np.sin(freqs).T.astype(np.float32)))


def _part_major(mat_t, free):
    """(HID_like, free) feature-major -> (128, KT_like, free) partition-major
    bf16 blocks: out[p, kt, f] = mat_t[kt*128 + p, f]."""
    r = mat_t.shape[0]
    return np.ascontiguousarray(
        mat_t.reshape(r // 128, 128, free).transpose(1, 0, 2).astype(NPBF16))


def _dup2(tab):
    """(128, N) f32 -> (128, 2, N) bf16 duplicated over dim 1."""
    return np.ascontiguousarray(
        np.repeat(tab[:, None, :], 2, axis=1).astype(NPBF16))


def _prep_in_maps(x, q_w, k_w, v_w, o_w, q_norm_w, k_norm_w):
    # q_w per pair: (128, KT, 512); feats = head_in_pair*256 + d
    qw_p = np.ascontiguousarray(
        q_w.reshape(NP, 2 * D, KT, 128).transpose(0, 3, 2, 1).astype(NPBF16))
    kwT = np.ascontiguousarray(k_w.T)                 # (HID, 1024)
    vwT = np.ascontiguousarray(v_w.T)
    kw_a = _part_major(kwT[:, :512], 512)
    kw_b = _part_major(kwT[:, 512:], 512)
    vw_a = _part_major(vwT[:, :512], 512)
    vw_b = _part_major(vwT[:, 512:], 512)
    # o_w: (128, 20, 16, 128); ow_p[p, mp, kk, f] = o_w[mp*128+f, kk*128+p]
    ow_p = np.ascontiguousarray(
        o_w.reshape(HID // 128, 128, 16, 128).transpose(3, 0, 2, 1)
        .astype(NPBF16))
    cosT, sinT = _rope_tables()                        # (128, L) each
    w1p = np.empty((128, 4), dtype=np.float32)
    w1p[:, 0] = 1.0 + q_norm_w[:128]
    w1p[:, 1] = 1.0 + q_norm_w[128:]
    w1p[:, 2] = 1.0 + k_norm_w[:128]
    w1p[:, 3] = 1.0 + k_norm_w[128:]
    ccol = (w1p ** -2).astype(NPBF16)

    kv_lo = L - W
    xkv_b = [_part_major(np.ascontiguousarray(x[b, kv_lo:, :].T), W)
             for b in range(B)]
    ck_t = _dup2(cosT[:, kv_lo:])
    sk_t = _dup2(sinT[:, kv_lo:])
    skn_t = _dup2(-sinT[:, kv_lo:])

    in_maps = []
    for c in range(NCORES):
        b, j = divmod(c, 4)
        rows = slice(j * CH, (j + 1) * CH)
        in_maps.append({
            "xq": _part_major(np.ascontiguousarray(x[b, rows, :].T), CH),
            "xkv": xkv_b[b],
            "qw": qw_p, "kwa": kw_a, "kwb": kw_b,
            "vwa": vw_a, "vwb": vw_b, "ow": ow_p,
            "cq": _dup2(cosT[:, rows]),
            "sq": _dup2(sinT[:, rows]),
            "sqn": _dup2(-sinT[:, rows]),
            "ck": ck_t, "sk": sk_t, "skn": skn_t,
            "w1p": w1p, "ccol": ccol,
            "onesc": np.ones((128, 1), dtype=NPBF16),
            "onesr": np.ones((1, 128), dtype=np.float32),
            "ident": np.eye(128, dtype=np.float32),
        })
    return in_maps


def kernel(x, mask, q_w, k_w, v_w, o_w, q_norm_w, k_norm_w):
    x = np.asarray(x, dtype=np.float32)
    q_w = np.asarray(q_w, dtype=np.float32)
    k_w = np.asarray(k_w, dtype=np.float32)
    v_w = np.asarray(v_w, dtype=np.float32)
    o_w = np.asarray(o_w, dtype=np.float32)
    q_norm_w = np.asarray(q_norm_w, dtype=np.float32)
    k_norm_w = np.asarray(k_norm_w, dtype=np.float32)

    nc = _get_nc()
    in_maps = _prep_in_maps(x, q_w, k_w, v_w, o_w, q_norm_w, k_norm_w)

    res = run_bass_kernel_spmd(nc, in_maps, list(range(NCORES)))
    _NC_CACHE["last_res"] = res

    out = np.empty((B, L, HID), dtype=np.float32)
    for c in range(NCORES):
        b, j = divmod(c, 4)
        out[b, j * CH:(j + 1) * CH, :] = res.results[c]["yT"].T
    return out



# revision 41
# speedup vs baseline: 1.0602x; 1.0104x over previous
"""Gemma3 sliding-window attention kernel for 8 Trainium2 NeuronCores.

Sharding: core c handles batch b = c//4, query-row chunk j = c%4 (512 rows).
The reference keeps only the LAST 512 key columns for every query row, so
each core computes k/v projections for rows 1536:2048 of its batch — all 4
kv heads locally (no collectives; the duplicated kv compute is cheaper than
the AllGather latency on hardware).

All matmul operands stream from HBM in bf16 (cast host-side); PSUM
accumulation is fp32 and softmax math stays fp32.

The attention works in HEAD PAIRS (each q-head pair shares one kv head, so
scores/softmax operands and the tanh scale coincide): DVE/ACT chain ops run
on [128, 2*512] pair tiles, halving the per-op overhead that dominates on
hardware. RMS normalization is deferred off the critical path: khat/qhat
hold rope((1+w)*raw); rs_k folds into the softcap tanh's per-partition
scale AP; rs_q is applied to qhat one pipeline step later from a
PE-broadcast row of sums of squares, with rsqrt done as a cubic seed + two
Newton steps (bf16 then fp32). Pipeline: step s issues qproj(pair s),
scores(s-2), attn_out(s-3).
"""

import numpy as np
import ml_dtypes

import concourse.bacc as bacc
import concourse.tile as tile
from concourse import mybir
from concourse.bass_utils import run_bass_kernel_spmd

F32 = mybir.dt.float32
F32R = mybir.dt.float32r
BF16 = mybir.dt.bfloat16
AF = mybir.ActivationFunctionType
OP = mybir.AluOpType

B, L, HID = 2, 2048, 2560
NH, NKV, D = 8, 4, 256
NP = NH // 2       # head pairs; pair p = heads (2p, 2p+1), kv head p
W = 512            # effective kv window (last W positions of the sequence)
CH = 512           # query rows per core
NCORES = 8
KT = HID // 128    # 20 contraction tiles for the projections
EPS = 1e-6
SOFTCAP = 50.0
SCALE = D ** -0.5
ROPE_BASE = 10000.0
NPBF16 = ml_dtypes.bfloat16
# tanh input scale c folded into the rs_k rsqrt: tanh((c*rs_k) * (rs_q*s))
C0 = SCALE / SOFTCAP
# cubic minimax seed for rsqrt on t in [0.3, 3.2] (rel err 4.9%), then two
# Newton steps (first bf16, second fp32) -> ~1e-4
RSQ_P3, RSQ_P2, RSQ_P1, RSQ_P0 = (-0.11751866, 0.81282722,
                                  -1.93345784, 2.24612936)


def _build(loop_n=None):
    nc = bacc.Bacc("TRN2", target_bir_lowering=False, debug=False,
                   num_devices=NCORES)
    xq_d = nc.dram_tensor("xq", [128, KT, CH], BF16, kind="ExternalInput").ap()
    xkv_d = nc.dram_tensor("xkv", [128, KT, W], BF16, kind="ExternalInput").ap()
    qw_d = nc.dram_tensor("qw", [NP, 128, KT, 2 * D], BF16,
                          kind="ExternalInput").ap()
    kwa_d = nc.dram_tensor("kwa", [128, KT, 512], BF16, kind="ExternalInput").ap()
    kwb_d = nc.dram_tensor("kwb", [128, KT, 512], BF16, kind="ExternalInput").ap()
    vwa_d = nc.dram_tensor("vwa", [128, KT, 512], BF16, kind="ExternalInput").ap()
    vwb_d = nc.dram_tensor("vwb", [128, KT, 512], BF16, kind="ExternalInput").ap()
    ow_d = nc.dram_tensor("ow", [128, HID // 128, 16, 128], BF16,
                          kind="ExternalInput").ap()
    # rope tables duplicated over the pair dim for [128, 2, *] chain ops
    cq = nc.dram_tensor("cq", [128, 2, CH], BF16, kind="ExternalInput").ap()
    sq = nc.dram_tensor("sq", [128, 2, CH], BF16, kind="ExternalInput").ap()
    sqn = nc.dram_tensor("sqn", [128, 2, CH], BF16, kind="ExternalInput").ap()
    ck = nc.dram_tensor("ck", [128, 2, W], BF16, kind="ExternalInput").ap()
    sk = nc.dram_tensor("sk", [128, 2, W], BF16, kind="ExternalInput").ap()
    skn = nc.dram_tensor("skn", [128, 2, W], BF16, kind="ExternalInput").ap()
    # columns: 1+qnw[:128], 1+qnw[128:], 1+knw[:128], 1+knw[128:]
    w1p = nc.dram_tensor("w1p", [128, 4], F32, kind="ExternalInput").ap()
    # (1+w)^-2 correction columns so sums of squares of the (1+w)-scaled
    # copies recover the raw-q/k norms; same column order as w1p
    ccol_d = nc.dram_tensor("ccol", [128, 4], BF16, kind="ExternalInput").ap()
    onesc_d = nc.dram_tensor("onesc", [128, 1], BF16, kind="ExternalInput").ap()
    onesr_d = nc.dram_tensor("onesr", [1, 128], F32R, kind="ExternalInput").ap()
    ident_d = nc.dram_tensor("ident", [128, 128], F32R, kind="ExternalInput").ap()
    yT = nc.dram_tensor("yT", [HID, CH], F32, kind="ExternalOutput").ap()

    NKC = 4
    CKT = KT // NKC

    with tile.TileContext(nc) as tc, \
            nc.allow_low_precision(reason='bf16 matmul operands'):
        with (
            tc.tile_pool(name="const", bufs=1) as pc,
            tc.tile_pool(name="px", bufs=2) as px,
            tc.tile_pool(name="pkw", bufs=2) as pkw,
            tc.tile_pool(name="pow", bufs=3) as pow_,
            tc.tile_pool(name="pkv", bufs=1) as pkv,
            tc.tile_pool(name="pq", bufs=1) as pq,
            tc.tile_pool(name="ptmp", bufs=2) as ptmp,
            tc.tile_pool(name="prow", bufs=1) as prow,
            tc.tile_pool(name="pexp", bufs=2) as pexp,
            tc.tile_pool(name="pout", bufs=2) as pout,
            tc.tile_pool(name="pdram", bufs=1, space="DRAM") as pdram,
            tc.tile_pool(name="pp", bufs=3, space="PSUM") as pp,
        ):
            import contextlib
            loop_ctx = tc.For_i(0, loop_n, 1) if loop_n else contextlib.nullcontext()
            # constants
            ones_col = pc.tile([128, 1], BF16, tag="onesc")
            nc.scalar.dma_start(out=ones_col, in_=onesc_d)
            ones_row = pc.tile([1, 128], F32R, tag="onesr")
            nc.scalar.dma_start(out=ones_row, in_=onesr_d)
            ident_sb = pc.tile([128, 128], F32R, tag="ident")
            nc.scalar.dma_start(out=ident_sb, in_=ident_d)
            ck_sb = pc.tile([128, 2, W], BF16, tag="c1")
            sk_sb = pc.tile([128, 2, W], BF16, tag="c2")
            skn_sb = pc.tile([128, 2, W], BF16, tag="c3")
            cq_sb = pc.tile([128, 2, CH], BF16, tag="c1")
            sq_sb = pc.tile([128, 2, CH], BF16, tag="c2")
            sqn_sb = pc.tile([128, 2, CH], BF16, tag="c3")
            w1p_sb = pc.tile([128, 4], F32, tag="w1p")
            nc.scalar.dma_start(out=w1p_sb, in_=w1p)
            ccol_sb = pc.tile([128, 4], BF16, tag="ccol")
            nc.scalar.dma_start(out=ccol_sb, in_=ccol_d)
            rsk_sb = pc.tile([128, 4 * NKV], F32, tag="rsk")
            # warm the ACT table set (exp/tanh/square/copy) while the first
            # weight DMAs are still in flight
            warm = pc.tile([128, 1], BF16, tag="warm")
            nc.scalar.activation(warm, ones_col, AF.Tanh)

            def rsqrt_sb(out_sb, t_sb, nfree, scale=1.0):
                """out = scale * t^-0.5 for SBUF f32 t (t in ~[0.3, 3.2]):
                cubic Horner seed + Newton iter in bf16, then one fp32
                Newton iter. y^2 goes through ACT Square."""
                z = ptmp.tile([128, nfree], BF16, tag="nwA", bufs=1)
                nc.vector.tensor_scalar(z, t_sb, RSQ_P3, RSQ_P2,
                                        op0=OP.mult, op1=OP.add)
                z2 = ptmp.tile([128, nfree], BF16, tag="nwB", bufs=1)
                nc.vector.scalar_tensor_tensor(z2, z, 0.0, t_sb,
                                               op0=OP.add, op1=OP.mult)
                z3 = ptmp.tile([128, nfree], BF16, tag="nwA", bufs=1)
                nc.vector.scalar_tensor_tensor(z3, z2, RSQ_P1, t_sb,
                                               op0=OP.add, op1=OP.mult)
                y = ptmp.tile([128, nfree], BF16, tag="nwB", bufs=1)
                nc.vector.tensor_scalar(y, z3, 1.0, RSQ_P0,
                                        op0=OP.mult, op1=OP.add)
                for it in range(2):
                    dt_ = BF16 if it == 0 else F32
                    sqy = ptmp.tile([128, nfree], dt_, tag="nwA", bufs=1,
                                    name=f"sqy{it}")
                    nc.vector.tensor_mul(sqy, y, y)
                    u = ptmp.tile([128, nfree], dt_, tag="nwC", bufs=1,
                                  name=f"nwu{it}")
                    nc.vector.tensor_mul(u, sqy, t_sb)
                    v = ptmp.tile([128, nfree], dt_, tag="nwA", bufs=1,
                                  name=f"nwv{it}")
                    nc.vector.tensor_scalar(v, u, -0.5, 1.5,
                                            op0=OP.mult, op1=OP.add)
                    if it == 0:
                        y1 = ptmp.tile([128, nfree], BF16, tag="nwD", bufs=1)
                        nc.vector.tensor_mul(y1, y, v)
                        y = y1
                    else:
                        nc.vector.scalar_tensor_tensor(out_sb, v, scale, y,
                                                       op0=OP.mult,
                                                       op1=OP.mult)

            def psum_evac(ps0p, ps1p, wcol0, wcol1, nfree, nm, d1=2):
                """Evacuate the projection PSUM pair through ACT copies that
                fold in the (1+w) scale; bf16 outputs let the rope chain run
                at the 2x DVE rate and free the PSUM banks after two ops."""
                c0 = ptmp.tile([128, d1, nfree], BF16, tag="qc0", bufs=2,
                               name=f"c0{nm}")
                c1 = ptmp.tile([128, d1, nfree], BF16, tag="qc1", bufs=2,
                               name=f"c1{nm}")
                nc.scalar.activation(c0, ps0p, AF.Copy, scale=wcol0)
                nc.scalar.activation(c1, ps1p, AF.Copy, scale=wcol1)
                return c0, c1

            def wrope_pair(c0, c1, h0, h1, cos2, sin2, nsin2, nfree, d1=2):
                """rope from the bf16 copies into the h0 (first-half) and
                h1 (second-half) destination APs."""
                a = ptmp.tile([128, d1, nfree], BF16, tag="ra", bufs=1)
                b2 = ptmp.tile([128, d1, nfree], BF16, tag="rb", bufs=1)
                nc.vector.tensor_mul(a, c0, cos2)
                nc.vector.tensor_mul(b2, c0, sin2)
                bn = ptmp.tile([128, d1, nfree], BF16, tag="nwA", bufs=1)
                a2 = ptmp.tile([128, d1, nfree], BF16, tag="nwB", bufs=1)
                nc.vector.tensor_mul(bn, c1, nsin2)
                nc.vector.tensor_mul(a2, c1, cos2)
                nc.vector.tensor_add(h0, a, bn)
                nc.vector.tensor_add(h1, a2, b2)

            with loop_ctx:
                khat = pkv.tile([128, 2 * NKV, W], BF16, tag="khat")
                v_sb = pkv.tile([128, 4, NKV * D], BF16, tag="v")
                qhat = pq.tile([128, 2 * NH, CH], BF16, tag="qhat")

                # ---- Phase 1: local kv projection, all 4 heads ----
                xkv_sb = px.tile([128, KT, W], BF16, tag="x")
                kw_sb = [pkw.tile([128, KT, 512], BF16, tag="w",
                                  name=f"kw{wv}") for wv in range(2)]
                CHUNKS = [(0, 1), (1, 2), (2, 4), (4, 8), (8, 14), (14, 20)]
                for lo, hi in CHUNKS:
                    sl = slice(lo, hi)
                    nc.sync.dma_start(out=kw_sb[0][:, sl, :], in_=kwa_d[:, sl, :])
                    nc.sync.dma_start(out=xkv_sb[:, sl, :], in_=xkv_d[:, sl, :])
                for c in range(NKC):
                    sl = slice(c * CKT, (c + 1) * CKT)
                    nc.sync.dma_start(out=kw_sb[1][:, sl, :], in_=kwb_d[:, sl, :])
                nc.scalar.dma_start(out=ck_sb, in_=ck)
                nc.scalar.dma_start(out=sk_sb, in_=sk)
                nc.scalar.dma_start(out=skn_sb, in_=skn)

                # k projection: 2 waves = 2 head pairs.  kps[m][:, gl, :] is
                # head gl's half m.  ssT accumulates transposed sums of
                # squares so rs_k becomes a per-partition tanh scale.
                ssT = pp.tile([128, 16], F32, tag="b1", name="ssT", bufs=2)
                for wv in range(2):
                    kps = [pp.tile([128, 2, W], F32, tag="b2",
                                   name=f"kps{wv}{m}") for m in range(2)]
                    for kt in range(KT):
                        for gl in range(2):
                            for m in range(2):
                                nc.tensor.matmul(
                                    kps[m][:, gl, :],
                                    kw_sb[wv][:, kt,
                                              gl * 256 + m * 128:
                                              gl * 256 + (m + 1) * 128],
                                    xkv_sb[:, kt, :],
                                    start=(kt == 0), stop=(kt == KT - 1))
                    kc = psum_evac(kps[0], kps[1], w1p_sb[:, 2:3],
                                   w1p_sb[:, 3:4], W, f"k{wv}")
                    sqk = [ptmp.tile([128, 2, W], BF16, tag="tA",
                                     name=f"sqk{wv}{m}") for m in range(2)]
                    for m in range(2):
                        nc.scalar.activation(sqk[m], kc[m], AF.Square)
                    for gl in range(2):
                        for mlk in range(4):
                            idx = (2 * wv + gl) * 4 + mlk
                            for m in range(2):
                                nc.tensor.matmul(
                                    ssT[:, idx:idx + 1],
                                    sqk[m][:, gl, mlk * 128:(mlk + 1) * 128],
                                    ccol_sb[:, 2 + m:3 + m],
                                    start=(m == 0), stop=(m == 1))
                    wrope_pair(kc[0], kc[1], khat[:, 4 * wv:4 * wv + 4:2, :],
                               khat[:, 4 * wv + 1:4 * wv + 4:2, :],
                               ck_sb, sk_sb, skn_sb, W)
                # rsk = c0 * (ssT/D + EPS)^-0.5 for all 4 kv heads at once
                tk = ptmp.tile([128, 16], F32, tag="tq", bufs=1, name="tk")
                nc.vector.tensor_scalar(tk, ssT, 1.0 / D, EPS,
                                        op0=OP.mult, op1=OP.add)
                rsqrt_sb(rsk_sb, tk, 16, scale=C0)

                # v projection: 2 waves x 2 heads
                vw_sb = [pkw.tile([128, KT, 512], BF16, tag="w",
                                  name=f"vw{wv}") for wv in range(2)]
                for wv, vd in ((0, vwa_d), (1, vwb_d)):
                    for c in range(NKC):
                        sl = slice(c * CKT, (c + 1) * CKT)
                        nc.sync.dma_start(out=vw_sb[wv][:, sl, :],
                                          in_=vd[:, sl, :])
                for wv in range(2):
                    vps = [pp.tile([128, 2, 512], F32, tag="b2",
                                   name=f"vps{wv}{mm2}") for mm2 in range(2)]
                    for kt in range(KT):
                        for m in range(4):
                            nc.tensor.matmul(
                                vps[m // 2][:, m % 2, :],
                                xkv_sb[:, kt, m * 128:(m + 1) * 128],
                                vw_sb[wv][:, kt, :],
                                start=(kt == 0), stop=(kt == KT - 1))
                    for mm2 in range(2):
                        # rows (2*mm2, 2*mm2+1) of the window chunk dim
                        nc.vector.tensor_copy(
                            v_sb[:, 2 * mm2:2 * mm2 + 2,
                                 wv * 512:(wv + 1) * 512],
                            vps[mm2])

                nc.sync.dma_start(out=cq_sb, in_=cq)
                nc.sync.dma_start(out=sq_sb, in_=sq)
                nc.sync.dma_start(out=sqn_sb, in_=sqn)

                # ---- Phase 2+3: pair pipeline  qproj(p) | scores(p-2) |
                #      attn_out(p-3) ----
                xq_sb = px.tile([128, KT, CH], BF16, tag="x")
                for c in range(NKC):
                    sl = slice(c * CKT, (c + 1) * CKT)
                    nc.sync.dma_start(out=xq_sb[:, sl, :], in_=xq_d[:, sl, :])
                aoT = px.tile([128, 2 * NH, CH], BF16, tag="x")

                qc_live = {}       # p -> [2 bf16 [128,2,CH] (1+w)-scaled copies]
                sqt_live = {}      # p -> [2 bf16 [128,2,CH] sq tiles]
                ssq_live = {}      # p -> transposed sums of squares [128,8] PSUM
                rsq8_live = {}     # p -> rs_q compact [128,8] f32
                rbq_live = {}      # p -> rs_q broadcast [128,2,CH] f32 PSUM
                exps_live = {}     # p -> exp tile [128, 4, 2, CH]
                dnrow_live = {}    # p -> [1, 2, CH] f32r
                qn_live = {}       # p -> normalized qhat [128, 4, CH] bf16
                qw_tiles = {}

                def qw_prefetch(p):
                    qw_t = pkw.tile([128, KT, 2 * D], BF16, tag="w",
                                    name=f"qwp{p}")
                    nc.sync.dma_start(out=qw_t, in_=qw_d[p])
                    qw_tiles[p] = qw_t

                def qproj_mms(p):
                    qw_t = qw_tiles.pop(p)
                    qps = [pp.tile([128, 2, CH], F32, tag="b2",
                                   name=f"qps{p}{m}") for m in range(2)]
                    for kt in range(KT):
                        for i in range(2):
                            for m in range(2):
                                nc.tensor.matmul(
                                    qps[m][:, i, :],
                                    qw_t[:, kt,
                                         i * 256 + m * 128:
                                         i * 256 + (m + 1) * 128],
                                    xq_sb[:, kt, :],
                                    start=(kt == 0), stop=(kt == KT - 1))
                    qc = psum_evac(qps[0], qps[1], w1p_sb[:, 0:1],
                                   w1p_sb[:, 1:2], CH, f"q{p}")
                    sqt = [ptmp.tile([128, 2, CH], BF16, tag="tA",
                                     name=f"sqt{p}{m}") for m in range(2)]
                    for m in range(2):
                        nc.scalar.activation(sqt[m], qc[m], AF.Square)
                    qc_live[p] = qc
                    sqt_live[p] = sqt

                def rope_chain(p):
                    qc = qc_live.pop(p)
                    wrope_pair(qc[0], qc[1],
                               qhat[:, 4 * p:4 * p + 4:2, :],
                               qhat[:, 4 * p + 1:4 * p + 4:2, :],
                               cq_sb, sq_sb, sqn_sb, CH)

                def ssq_mms(p):
                    """Transposed sums of squares: [128(row), 8] where
                    col idx = i*4 + ch covers (head-in-pair i, 128-row
                    chunk ch)."""
                    sqt = sqt_live.pop(p)
                    ssq = pp.tile([128, 8], F32, tag="b1", bufs=2,
                                  name=f"ssq{p}")
                    for i in range(2):
                        for ch in range(4):
                            idx = i * 4 + ch
                            for m in range(2):
                                nc.tensor.matmul(
                                    ssq[:, idx:idx + 1],
                                    sqt[m][:, i, ch * 128:(ch + 1) * 128],
                                    ccol_sb[:, m:m + 1],
                                    start=(m == 0), stop=(m == 1))
                    ssq_live[p] = ssq

                def newton_c(p):
                    """rs_q on the compact [128,8] layout (cheap on DVE)."""
                    ssq = ssq_live.pop(p)
                    t8 = ptmp.tile([128, 8], F32, tag="tq", bufs=1,
                                   name=f"t8{p}")
                    nc.vector.tensor_scalar(t8, ssq, 1.0 / D, EPS,
                                            op0=OP.mult, op1=OP.add)
                    rsq8 = ptmp.tile([128, 8], F32R, tag="rbB", bufs=2,
                                     name=f"rsq8{p}")
                    rsqrt_sb(rsq8, t8, 8)
                    rsq8_live[p] = rsq8

                def trans_bcast(p):
                    """Broadcast compact rs_q to [128, 2, CH] PSUM via
                    stride-0 lhsT x identity: out[d, i, ch*128+j] =
                    rsq8[j, i*4+ch] for every partition d."""
                    rsq8r = rsq8_live.pop(p)
                    rbq_ps = pp.tile([128, 2, CH], F32, tag="b2",
                                     name=f"rbq{p}")
                    for i in range(2):
                        for ch in range(4):
                            idx = i * 4 + ch
                            nc.tensor.matmul(
                                rbq_ps[:, i, ch * 128:(ch + 1) * 128],
                                rsq8r[:, idx:idx + 1].to_broadcast([128, 128]),
                                ident_sb,
                                start=True, stop=True)
                    rbq_live[p] = rbq_ps

                def qnorm_mul(p):
                    rbq = rbq_live.pop(p)
                    qn = pq.tile([128, 4, CH], BF16, tag="qhatn", bufs=2,
                                 name=f"qhatn{p}")
                    for i in range(2):
                        for dk in range(2):
                            nc.vector.tensor_mul(
                                qn[:, 2 * i + dk, :],
                                qhat[:, 4 * p + 2 * i + dk, :],
                                rbq[:, i, :])
                    qn_live[p] = qn

                def sps_softmax(p):
                    g = p
                    qn = qn_live.pop(p)
                    exps = pexp.tile([128, 4, 2, CH], BF16, tag="exps",
                                     name=f"exps{p}")
                    for mlk in range(4):
                        sps = pp.tile([128, 2, CH], F32, tag="b2",
                                      name=f"sps{p}{mlk}")
                        for i in range(2):
                            for dk in range(2):
                                nc.tensor.matmul(
                                    sps[:, i, :],
                                    khat[:, 2 * g + dk,
                                         mlk * 128:(mlk + 1) * 128],
                                    qn[:, 2 * i + dk, :],
                                    start=(dk == 0), stop=(dk == 1))
                        nc.scalar.activation(
                            sps, sps, AF.Tanh,
                            scale=rsk_sb[:, g * 4 + mlk:g * 4 + mlk + 1])
                        nc.scalar.activation(exps[:, mlk, :, :], sps, AF.Exp,
                                             scale=SOFTCAP)
                    exps_live[p] = exps

                def dn_part(p):
                    exps = exps_live[p]
                    dn_ps = pp.tile([1, 2, CH], F32, tag="b2",
                                    name=f"dn{p}")
                    for i in range(2):
                        for mlk in range(4):
                            nc.tensor.matmul(dn_ps[:, i, :], ones_col,
                                             exps[:, mlk, i, :],
                                             start=(mlk == 0),
                                             stop=(mlk == 3))
                    dnrow = prow.tile([1, 2, CH], F32R, tag="drow",
                                      name=f"dnrow{p}")
                    nc.scalar.copy(dnrow, dn_ps)
                    dnrow_live[p] = dnrow

                def ops_fin(p):
                    g = p
                    exps = exps_live.pop(p)
                    opst = []
                    for dh in range(2):
                        ops = pp.tile([128, 2, CH], F32, tag="b2",
                                      name=f"ops{p}{dh}")
                        for i in range(2):
                            for klk in range(4):
                                nc.tensor.matmul(
                                    ops[:, i, :],
                                    v_sb[:, klk,
                                         g * 256 + dh * 128:
                                         g * 256 + dh * 128 + 128],
                                    exps[:, klk, i, :],
                                    start=(klk == 0), stop=(klk == 3))
                        opst.append(ops)
                    rbat_ps = pp.tile([128, 2, CH], F32, tag="b2",
                                      name=f"rbat{p}")
                    dnrow = dnrow_live.pop(p)
                    for i in range(2):
                        nc.tensor.matmul(rbat_ps[:, i, :], ones_row,
                                         dnrow[:, i, :], start=True,
                                         stop=True)
                    rbat = ptmp.tile([128, 2, CH], F32, tag="rbC", bufs=1,
                                     name=f"rbat{p}")
                    nc.vector.reciprocal_approx_fast(rbat, rbat_ps)
                    for dh in range(2):
                        # aoT slots {4p+dh, 4p+2+dh}
                        nc.vector.tensor_mul(
                            aoT[:, 4 * p + dh:4 * p + dh + 3:2, :],
                            opst[dh], rbat)

                qw_prefetch(0)
                for s in range(NP + 3):
                    if 0 <= s - 3 < NP:
                        dn_part(s - 3)
                    if 0 <= s - 1 < NP:
                        newton_c(s - 1)
                    if 0 <= s - 3 < NP:
                        ops_fin(s - 3)
                    if 0 <= s - 1 < NP:
                        trans_bcast(s - 1)
                        qnorm_mul(s - 1)
                    if 0 <= s - 2 < NP:
                        sps_softmax(s - 2)
                    if s + 1 < NP:
                        qw_prefetch(s + 1)
                    if s < NP:
                        qproj_mms(s)
                    if s < NP:
                        ssq_mms(s)
                    if s < NP:
                        rope_chain(s)

                # ---- Phase 4: o projection (outputs transposed: yT) ----
                for mp2 in range(HID // 256):
                    yps = pp.tile([128, 2, CH], F32, tag="b2",
                                  name=f"yps{mp2}")
                    for j in range(2):
                        mp = 2 * mp2 + j
                        owc = pow_.tile([128, 16, 128], BF16, tag="ow",
                                        name=f"ow{mp}")
                        nc.sync.dma_start(out=owc, in_=ow_d[:, mp, :, :])
                        for kk in range(16):
                            nc.tensor.matmul(yps[:, j, :], owc[:, kk, :],
                                             aoT[:, kk, :],
                                             start=(kk == 0), stop=(kk == 15))
                    yst = pout.tile([128, 2, CH], F32, tag="yst")
                    nc.scalar.copy(yst, yps)
                    for j in range(2):
                        nc.sync.dma_start(
                            out=yT[(2 * mp2 + j) * 128:
                                   (2 * mp2 + j + 1) * 128, :],
                            in_=yst[:, j, :])

    nc.compile()

    return nc


_NC_CACHE = {}


def _get_nc():
    if "nc" not in _NC_CACHE:
        _NC_CACHE["nc"] = _build()
    return _NC_CACHE["nc"]


def _rope_tables():
    inv_freq = 1.0 / (ROPE_BASE ** (np.arange(0, D, 2, dtype=np.float32) / D))
    t = np.arange(L, dtype=np.float32)
    freqs = np.outer(t, inv_freq)                     # (L, 128)
    return (np.ascontiguousarray(np.cos(freqs).T.astype(np.float32)),
            np.ascontiguousarray(np.sin(freqs).T.astype(np.float32)))


def _part_major(mat_t, free):
    """(HID_like, free) feature-major -> (128, KT_like, free) partition-major
    bf16 blocks: out[p, kt, f] = mat_t[kt*128 + p, f]."""
    r = mat_t.shape[0]
    return np.ascontiguousarray(
        mat_t.reshape(r // 128, 128, free).transpose(1, 0, 2).astype(NPBF16))


def _dup2(tab):
    """(128, N) f32 -> (128, 2, N) bf16 duplicated over dim 1."""
    return np.ascontiguousarray(
        np.repeat(tab[:, None, :], 2, axis=1).astype(NPBF16))


def _prep_in_maps(x, q_w, k_w, v_w, o_w, q_norm_w, k_norm_w):
    # q_w per pair: (128, KT, 512); feats = head_in_pair*256 + d
    qw_p = np.ascontiguousarray(
        q_w.reshape(NP, 2 * D, KT, 128).transpose(0, 3, 2, 1).astype(NPBF16))
    kwT = np.ascontiguousarray(k_w.T)                 # (HID, 1024)
    vwT = np.ascontiguousarray(v_w.T)
    kw_a = _part_major(kwT[:, :512], 512)
    kw_b = _part_major(kwT[:, 512:], 512)
    vw_a = _part_major(vwT[:, :512], 512)
    vw_b = _part_major(vwT[:, 512:], 512)
    # o_w: (128, 20, 16, 128); ow_p[p, mp, kk, f] = o_w[mp*128+f, kk*128+p]
    ow_p = np.ascontiguousarray(
        o_w.reshape(HID // 128, 128, 16, 128).transpose(3, 0, 2, 1)
        .astype(NPBF16))
    cosT, sinT = _rope_tables()                        # (128, L) each
    w1p = np.empty((128, 4), dtype=np.float32)
    w1p[:, 0] = 1.0 + q_norm_w[:128]
    w1p[:, 1] = 1.0 + q_norm_w[128:]
    w1p[:, 2] = 1.0 + k_norm_w[:128]
    w1p[:, 3] = 1.0 + k_norm_w[128:]
    ccol = (w1p ** -2).astype(NPBF16)

    kv_lo = L - W
    xkv_b = [_part_major(np.ascontiguousarray(x[b, kv_lo:, :].T), W)
             for b in range(B)]
    ck_t = _dup2(cosT[:, kv_lo:])
    sk_t = _dup2(sinT[:, kv_lo:])
    skn_t = _dup2(-sinT[:, kv_lo:])

    in_maps = []
    for c in range(NCORES):
        b, j = divmod(c, 4)
        rows = slice(j * CH, (j + 1) * CH)
        in_maps.append({
            "xq": _part_major(np.ascontiguousarray(x[b, rows, :].T), CH),
            "xkv": xkv_b[b],
            "qw": qw_p, "kwa": kw_a, "kwb": kw_b,
            "vwa": vw_a, "vwb": vw_b, "ow": ow_p,
            "cq": _dup2(cosT[:, rows]),
            "sq": _dup2(sinT[:, rows]),
            "sqn": _dup2(-sinT[:, rows]),
            "ck": ck_t, "sk": sk_t, "skn": skn_t,
            "w1p": w1p, "ccol": ccol,
            "onesc": np.ones((128, 1), dtype=NPBF16),
            "onesr": np.ones((1, 128), dtype=np.float32),
            "ident": np.eye(128, dtype=np.float32),
        })
    return in_maps


def kernel(x, mask, q_w, k_w, v_w, o_w, q_norm_w, k_norm_w):
    x = np.asarray(x, dtype=np.float32)
    q_w = np.asarray(q_w, dtype=np.float32)
    k_w = np.asarray(k_w, dtype=np.float32)
    v_w = np.asarray(v_w, dtype=np.float32)
    o_w = np.asarray(o_w, dtype=np.float32)
    q_norm_w = np.asarray(q_norm_w, dtype=np.float32)
    k_norm_w = np.asarray(k_norm_w, dtype=np.float32)

    nc = _get_nc()
    in_maps = _prep_in_maps(x, q_w, k_w, v_w, o_w, q_norm_w, k_norm_w)

    res = run_bass_kernel_spmd(nc, in_maps, list(range(NCORES)))
    _NC_CACHE["last_res"] = res

    out = np.empty((B, L, HID), dtype=np.float32)
    for c in range(NCORES):
        b, j = divmod(c, 4)
        out[b, j * CH:(j + 1) * CH, :] = res.results[c]["yT"].T
    return out

